# revision 16
# baseline (speedup 1.0000x reference)
"""Trainium2 Bass kernel for nn_MoEST_Plus (MoE spatial transformer).

Sharding: data-parallel over the N (spot) axis. Each of the 8 cores
receives a token-rotated copy of the full inputs so that its OWN 512
tokens sit at positions 0..511; the encoder (z) is computed for all 4096
tokens on every core (attention needs full K/V; replication avoids
collectives), while attention-queries / MoE / decoders run only on the
core's own 512 tokens. Outputs are gathered and de-interleaved on host.

All on-device activations use a feature-major ("transposed", [feat, tok])
or token-major ([tok, feat]) layout chosen per-op so every matmul
contraction sits on the partition axis.
"""

import os
import sys

import numpy as np

for _p in ("/opt/trn_rl_repo",):
    if os.path.isdir(_p) and _p not in sys.path:
        sys.path.insert(0, _p)

import concourse.bacc as bacc
import concourse.mybir as mybir
import concourse.tile as tile
from concourse import masks
from concourse.bass_utils import run_bass_kernel_spmd

AF = mybir.ActivationFunctionType
ALU = mybir.AluOpType
F32 = mybir.dt.float32

N = 4096          # tokens (spots)
DU = 1024         # dim_uni
D = 256           # dim_hidden
G = 2000          # genes
E = 4             # experts
H = 4             # heads
HD = 64           # head dim
MAP = 64          # fourier mapping size
NCORES = 8
S = N // NCORES   # own tokens per core = 512
QS = S // 128     # 128-token q tiles = 4
TT = N // 512     # 512-token tiles = 8
KT = N // 128     # 128-token key tiles = 32
PI = float(np.pi)


def _build_body(nc, tc, io):
    """Emit the whole model as one Tile program."""
    from contextlib import ExitStack

    es = ExitStack()

    def pool(name, bufs, space="SBUF"):
        return tc.tile_pool(name=name, bufs=bufs, space=space)

    # ---------------- constants / small params ----------------
    cpool = es.enter_context(tc.tile_pool(name="consts", bufs=1))
    ident = cpool.tile([128, 128], F32, tag="ident", name="ident")
    masks.make_identity(nc, ident[:])
    ones1 = cpool.tile([1, 128], F32, tag="ones1", name="ones1")
    nc.gpsimd.memset(ones1[:], 1.0)
    pi_c = cpool.tile([128, 1], F32, tag="pi_c", name="pi_c")
    nc.gpsimd.memset(pi_c[:], PI)
    eps5_c = cpool.tile([128, 1], F32, tag="eps5_c", name="eps5_c")
    nc.gpsimd.memset(eps5_c[:], 1e-5)

    def dma(dst, src):
        nc.sync.dma_start(dst, src)

    def row(dram_1d, n, tag):
        """Load a 1-D DRAM vector as a [1, n] SBUF row."""
        t = cpool.tile([1, n], F32, tag=tag)
        dma(t[:], dram_1d.rearrange("(o x) -> o x", o=1))
        return t

    def col(dram_1d, lo, n, tag):
        """Load dram_1d[lo:lo+n] as an [n, 1] SBUF column."""
        t = cpool.tile([n, 1], F32, tag=tag)
        dma(t[:], dram_1d[lo:lo + n].rearrange("(p o) -> p o", o=1))
        return t

    # per-partition bias columns for feature-major psum->sbuf copies
    bimg_c = [col(io["b_img"], dd * 128, 128, f"bimg{dd}") for dd in range(2)]
    bpos_c = [col(io["b_pos"], dd * 128, 128, f"bpos{dd}") for dd in range(2)]
    bz_c = []
    for dd in range(2):
        t = cpool.tile([128, 1], F32, tag=f"bz{dd}", name=f"bz{dd}")
        nc.vector.tensor_add(t[:], bimg_c[dd][:], bpos_c[dd][:])
        bz_c.append(t)
    bk_c = [col(io["bk"], dd * 128, 128, f"bk{dd}") for dd in range(2)]
    bq_c = [col(io["bq"], dd * 128, 128, f"bq{dd}") for dd in range(2)]
    bf1_c = col(io["bf1"], 0, 64, "bf1")

    # bias rows for ones-row matmul bias adds (token-major psums)
    bv_row = row(io["bv"], D, "bvr")
    bo_row = row(io["bo"], D, "bor")
    brt_row = row(io["b_router"], E, "brtr")
    bg1_row = row(io["bg1"], D, "bg1r")
    bf2_row = row(io["bf2"], 1, "bf2r")

    # LN gamma/beta broadcast tiles [128, 256] via PE outer-product
    ln_rows = {k: row(io[k], D, k + "r") for k in ("ln1_g", "ln1_b", "lng_g", "lng_b")}
    ln_b = {}
    with pool("lnb_ps", 2, space="PSUM") as lnps:
        for k, r in ln_rows.items():
            ps = lnps.tile([128, D], F32, tag="lnb", name="lnb")
            nc.tensor.matmul(ps[:], lhsT=ones1[:], rhs=r[:], start=True, stop=True)
            t = cpool.tile([128, D], F32, tag=k + "b")
            nc.vector.tensor_copy(t[:], ps[:])
            ln_b[k] = t

    # eps tile for theta += 1e-6 (odd output columns)
    eps_t = cpool.tile([128, 512], F32, tag="epst", name="epst")
    nc.gpsimd.memset(eps_t[:], 0.0)
    nc.gpsimd.memset(eps_t[:].rearrange("p (g two) -> p g two", two=2)[:, :, 1:2], 1e-6)

    # B_fourier [3, 64]
    bfour = cpool.tile([3, MAP], F32, tag="bfour", name="bfour")
    dma(bfour[:], io["B_fourier"][:])

    # ---------------- persistent activation tiles ----------------
    act_pool = es.enter_context(tc.tile_pool(name="acts", bufs=1))
    z_own = act_pool.tile([128, QS * D], F32, tag="z_own", name="z_own")      # token-major, 4 qs blocks
    z2_sb = act_pool.tile([128, QS * D], F32, tag="z2", name="z2")         # post-LN1 token-major
    z2T = [act_pool.tile([128, S], F32, tag=f"z2T{dd}", name=f"z2T{dd}") for dd in range(2)]
    z3_sb = act_pool.tile([128, QS * D], F32, tag="z3", name="z3")         # post-MoE token-major
    gate_sb = act_pool.tile([128, QS * E], F32, tag="gate", name="gate")     # router gates per qs block

    # =====================================================================
    # Phase 1+2: encoder (z_T full) then k/v/q projections
    # =====================================================================
    zT_cm = tc.tile_pool(name="zT_pool", bufs=1)
    zT_pool = zT_cm.__enter__()
    zT = [zT_pool.tile([128, N], F32, tag=f"zT{dd}", name=f"zT{dd}") for dd in range(2)]

    encw_cm = tc.tile_pool(name="enc_sb", bufs=1)
    enc_sb = encw_cm.__enter__()
    with pool("vis_sb", 6) as vis_pool, \
         pool("four_sb", 2) as four_pool, \
         pool("enc_ps", 4, space="PSUM") as enc_ps, \
         pool("fours_ps", 2, space="PSUM") as four_ps:
        wimg = enc_sb.tile([128, 8 * D], F32, tag="wimg", name="wimg")
        for kk in range(8):
            dma(wimg[:, kk * D:(kk + 1) * D], io["W_img"][kk * 128:(kk + 1) * 128, :])
        wpos = enc_sb.tile([128, D], F32, tag="wpos", name="wpos")
        dma(wpos[:], io["W_pos"][:])

        for tt in range(TT):
            t0 = tt * 512
            # fourier features for this token tile
            pos_sb = four_pool.tile([3, 512], F32, tag="pos", name="pos")
            dma(pos_sb[:], io["posT"][:, t0:t0 + 512])
            xp = four_ps.tile([MAP, 512], F32, tag="xp", name="xp")
            nc.tensor.matmul(xp[:], lhsT=bfour[:], rhs=pos_sb[:], start=True, stop=True)
            # range-reduce via round-to-nearest (magic constant):
            # sin(2*pi*t) = Sin(2*pi*(t - round(t))), arg in [-pi, pi]
            MAGIC = 12582912.0  # 1.5 * 2**23
            r1 = four_pool.tile([MAP, 512], F32, tag="r1", name="r1")
            nc.vector.tensor_scalar(r1[:], xp[:], MAGIC, -MAGIC, ALU.add, ALU.add)
            fr = four_pool.tile([MAP, 512], F32, tag="fr", name="fr")
            nc.vector.tensor_tensor(fr[:], xp[:], r1[:], ALU.subtract)
            t2 = four_pool.tile([MAP, 512], F32, tag="t2", name="t2")
            nc.vector.tensor_scalar(t2[:], xp[:], 0.25, None, ALU.add)
            r2 = four_pool.tile([MAP, 512], F32, tag="r2", name="r2")
            nc.vector.tensor_scalar(r2[:], t2[:], MAGIC, -MAGIC, ALU.add, ALU.add)
            fr2 = four_pool.tile([MAP, 512], F32, tag="fr2", name="fr2")
            nc.vector.tensor_tensor(fr2[:], t2[:], r2[:], ALU.subtract)
            four = four_pool.tile([128, 512], F32, tag="four", name="four")
            nc.scalar.activation(four[0:MAP, :], fr[:], AF.Sin, scale=2 * PI)
            nc.scalar.activation(four[MAP:128, :], fr2[:], AF.Sin, scale=2 * PI)

            # vis K-tiles for this token tile
            vtiles = []
            for kk in range(8):
                vt = vis_pool.tile([128, 512], F32, tag="vis", name="vis")
                dma(vt[:], io["visT"][kk * 128:(kk + 1) * 128, t0:t0 + 512])
                vtiles.append(vt)

            for dd in range(2):
                zps = enc_ps.tile([128, 512], F32, tag="zps", name="zps")
                for kk in range(8):
                    nc.tensor.matmul(
                        zps[:], lhsT=wimg[:, kk * D + dd * 128:kk * D + dd * 128 + 128],
                        rhs=vtiles[kk][:], start=(kk == 0), stop=False)
                nc.tensor.matmul(
                    zps[:], lhsT=wpos[:, dd * 128:dd * 128 + 128],
                    rhs=four[:], start=False, stop=True)
                # psum -> sbuf with bias (b_img + b_pos)
                nc.scalar.add(zT[dd][:, t0:t0 + 512], zps[:], add=bz_c[dd][:])

    encw_cm.__exit__(None, None, None)

    attnio_cm = tc.tile_pool(name="attn_io", bufs=1)
    attn_io = attnio_cm.__enter__()
    kT = [attn_io.tile([128, N], F32, tag=f"kT{dd}", name=f"kT{dd}") for dd in range(2)]
    v_sb = attn_io.tile([128, KT * (H * (HD + 1))], F32, tag="v1", name="v1")
    qT = [attn_io.tile([128, S], F32, tag=f"qT{dd}", name=f"qT{dd}") for dd in range(2)]
    nc.gpsimd.memset(
        v_sb[:].rearrange("p (t h c) -> p (t h) c", h=H, c=HD + 1)[:, :, HD:HD + 1], 1.0
    )

    if True:
        # ---- k/v/q projections + z_own transpose (needs zT) ----
        with pool("kvq_sb", 1) as kvq_sb, \
             pool("kvq_ps", 3, space="PSUM") as kvq_ps, \
             pool("tp_ps", 2, space="PSUM") as tp_ps:
            wk = kvq_sb.tile([128, 2 * D], F32, tag="wk", name="wk")
            wq = kvq_sb.tile([128, 2 * D], F32, tag="wq", name="wq")
            wv = kvq_sb.tile([128, 2 * D], F32, tag="wv", name="wv")
            for kk in range(2):
                dma(wk[:, kk * D:(kk + 1) * D], io["Wk"][kk * 128:(kk + 1) * 128, :])
                dma(wq[:, kk * D:(kk + 1) * D], io["Wq"][kk * 128:(kk + 1) * 128, :])
                dma(wv[:, kk * D:(kk + 1) * D], io["Wv"][kk * 128:(kk + 1) * 128, :])

            # k_T (feature-major, all tokens)
            for dd in range(2):
                for tt in range(TT):
                    t0 = tt * 512
                    kps = kvq_ps.tile([128, 512], F32, tag="kps", name="kps")
                    for kk in range(2):
                        nc.tensor.matmul(
                            kps[:], lhsT=wk[:, kk * D + dd * 128:kk * D + dd * 128 + 128],
                            rhs=zT[kk][:, t0:t0 + 512], start=(kk == 0), stop=(kk == 1))
                    nc.scalar.add(kT[dd][:, t0:t0 + 512], kps[:], add=bk_c[dd][:])

            # q_T (feature-major, own tokens = first 512)
            for dd in range(2):
                qps = kvq_ps.tile([128, 512], F32, tag="kps", name="kps")
                for kk in range(2):
                    nc.tensor.matmul(
                        qps[:], lhsT=wq[:, kk * D + dd * 128:kk * D + dd * 128 + 128],
                        rhs=zT[kk][:, 0:512], start=(kk == 0), stop=(kk == 1))
                nc.scalar.add(qT[dd][:], qps[:], add=bq_c[dd][:])

            # v (token-major, all tokens) with bias via ones-row
            for kt in range(KT):
                p0 = kt * 128
                vps = kvq_ps.tile([128, D], F32, tag="vps", name="vps")
                for kk in range(2):
                    nc.tensor.matmul(
                        vps[:], lhsT=zT[kk][:, p0:p0 + 128],
                        rhs=wv[:, kk * D:(kk + 1) * D], start=(kk == 0), stop=False)
                nc.tensor.matmul(vps[:], lhsT=ones1[:], rhs=bv_row[:],
                                 start=False, stop=True)
                dst = v_sb[:, kt * H * (HD + 1):(kt + 1) * H * (HD + 1)]
                dst = dst.rearrange("p (h c) -> p h c", c=HD + 1)[:, :, 0:HD]
                nc.vector.tensor_copy(dst, vps[:].rearrange("p (h c) -> p h c", c=HD))

            # z_own (token-major) from zT
            for qs in range(QS):
                for dd in range(2):
                    tp = tp_ps.tile([128, 128], F32, tag="tp", name="tp")
                    nc.tensor.transpose(tp[:], zT[dd][:, qs * 128:(qs + 1) * 128], ident[:])
                    nc.vector.tensor_copy(
                        z_own[:, qs * D + dd * 128:qs * D + dd * 128 + 128], tp[:])

    # =====================================================================
    # Phase 3: attention (own 512 queries over all 4096 keys)
    # =====================================================================
    with pool("attn_sb", 1) as attn_sb, \
         pool("p_sb", 4) as p_pool:
        wo_sb = attn_sb.tile([HD, H * D], F32, tag="wo", name="wo")
        for h in range(H):
            dma(wo_sb[:, h * D:(h + 1) * D], io["Wo"][h * HD:(h + 1) * HD, :])
        grad_sb = attn_sb.tile([1, S], F32, tag="grad", name="grad")
        dma(grad_sb[:], io["gradT"][:])
        wr_a = attn_sb.tile([128, E], F32, tag="wra", name="wra")
        wr_b = attn_sb.tile([128, E], F32, tag="wrb", name="wrb")
        wr_c = attn_sb.tile([1, E], F32, tag="wrc", name="wrc")
        dma(wr_a[:], io["W_router"][0:128, :])
        dma(wr_b[:], io["W_router"][128:256, :])
        dma(wr_c[:], io["W_router"][256:257, :])
        ctxn = [attn_sb.tile([HD, S], F32, tag=f"ctxn{h}", name=f"ctxn{h}") for h in range(H)]
        heads_ps_cm = [pool("s_ps", 3, space="PSUM"), pool("ctx_ps", 2, space="PSUM")]
        s_ps = heads_ps_cm[0].__enter__()
        ctx_ps_pool = heads_ps_cm[1].__enter__()
        for h in range(H):
            dd, off = h // 2, (h % 2) * HD
            ctx_ps = ctx_ps_pool.tile([HD + 1, S], F32, tag="ctx", name="ctx")
            for kt in range(KT):
                sps = s_ps.tile([128, S], F32, tag="sps", name="sps")
                nc.tensor.matmul(
                    sps[:], lhsT=kT[dd][off:off + HD, kt * 128:(kt + 1) * 128],
                    rhs=qT[dd][off:off + HD, :], start=True, stop=True)
                pt = p_pool.tile([128, S], F32, tag="pt", name="pt")
                nc.scalar.activation(pt[:], sps[:], AF.Exp, scale=1.0 / 8.0)
                c0 = kt * H * (HD + 1) + h * (HD + 1)
                nc.tensor.matmul(ctx_ps[:], lhsT=v_sb[:, c0:c0 + HD + 1], rhs=pt[:],
                                 start=(kt == 0), stop=(kt == KT - 1))
            # normalize by softmax denominator (row HD of ctx_ps)
            rd = attn_sb.tile([1, S], F32, tag="rd", name="rd")
            nc.vector.reciprocal(rd[:], ctx_ps[HD:HD + 1, :])
            rb_ps = s_ps.tile([HD, S], F32, tag="sps", name="sps")
            nc.tensor.matmul(rb_ps[:], lhsT=ones1[0:1, 0:HD], rhs=rd[:],
                             start=True, stop=True)
            rb = attn_sb.tile([HD, S], F32, tag="rb", name="rb")
            nc.vector.tensor_copy(rb[:], rb_ps[:])
            nc.vector.tensor_tensor(ctxn[h][:], ctx_ps[0:HD, :], rb[:], ALU.mult)

        ctx_ps_pool = heads_ps_cm[1].__exit__(None, None, None)
        s_ps = heads_ps_cm[0].__exit__(None, None, None)

        # attn_out = sum_h ctxn_h @ Wo_h + bo ; residual + LN1 ; router+gates
        with pool("wo_ps", 2, space="PSUM") as wo_ps, \
             pool("ln_sb", 1) as ln_sb, \
             pool("r_ps", 2, space="PSUM") as r_ps, \
             pool("tp2_ps", 2, space="PSUM") as tp2_ps:
            xs, stats = [], []
            for qs in range(QS):
                aps = wo_ps.tile([128, D], F32, tag="aps", name="aps")
                for h in range(H):
                    dd, off = h // 2, (h % 2) * HD
                    nc.tensor.matmul(
                        aps[:], lhsT=ctxn[h][:, qs * 128:(qs + 1) * 128],
                        rhs=wo_sb[:, h * D:(h + 1) * D],
                        start=(h == 0), stop=False)
                nc.tensor.matmul(aps[:], lhsT=ones1[:], rhs=bo_row[:],
                                 start=False, stop=True)
                # x = z_own + attn_out ; LN stats
                x = ln_sb.tile([128, D], F32, tag=f"x{qs}", name=f"x{qs}")
                nc.vector.tensor_tensor(x[:], z_own[:, qs * D:(qs + 1) * D], aps[:], ALU.add)
                sum_x = ln_sb.tile([128, 1], F32, tag=f"sx{qs}", name=f"sx{qs}")
                scratch = ln_sb.tile([128, D], F32, tag="lnscr", name="lnscr")
                nc.scalar.activation(scratch[:], x[:], AF.Copy, accum_out=sum_x[:])
                sum_x2 = ln_sb.tile([128, 1], F32, tag=f"sx2{qs}", name=f"sx2{qs}")
                nc.scalar.activation(scratch[:], x[:], AF.Square, accum_out=sum_x2[:])
                xs.append(x)
                stats.append((sum_x, sum_x2))
            # batched sqrt (one ACT table visit), Newton-refined
            rstds = []
            for qs in range(QS):
                sum_x, sum_x2 = stats[qs]
                m = ln_sb.tile([128, 1], F32, tag=f"m{qs}", name=f"m{qs}")
                nc.vector.tensor_scalar(m[:], sum_x[:], 1.0 / D, None, ALU.mult)
                msq = ln_sb.tile([128, 1], F32, tag=f"msq{qs}", name=f"msq{qs}")
                nc.vector.tensor_tensor(msq[:], m[:], m[:], ALU.mult)
                var = ln_sb.tile([128, 1], F32, tag=f"var{qs}", name=f"var{qs}")
                nc.vector.tensor_scalar(var[:], sum_x2[:], 1.0 / D, msq[:], ALU.mult, ALU.subtract)
                sd = ln_sb.tile([128, 1], F32, tag=f"sd{qs}", name=f"sd{qs}")
                nc.scalar.activation(sd[:], var[:], AF.Sqrt, bias=eps5_c[:])
                # one Newton step: sd1 = 0.5*(sd + (var+eps)/sd)
                veps = ln_sb.tile([128, 1], F32, tag=f"veps{qs}", name=f"veps{qs}")
                nc.vector.tensor_scalar(veps[:], var[:], 1e-5, None, ALU.add)
                rsd = ln_sb.tile([128, 1], F32, tag=f"rsd{qs}", name=f"rsd{qs}")
                nc.vector.reciprocal(rsd[:], sd[:])
                q1 = ln_sb.tile([128, 1], F32, tag=f"q1{qs}", name=f"q1{qs}")
                nc.vector.tensor_tensor(q1[:], veps[:], rsd[:], ALU.mult)
                sd1 = ln_sb.tile([128, 1], F32, tag=f"sd1{qs}", name=f"sd1{qs}")
                nc.vector.tensor_scalar(sd1[:], q1[:], sd[:], 0.5, ALU.add, ALU.mult)
                rstd = ln_sb.tile([128, 1], F32, tag=f"rstd{qs}", name=f"rstd{qs}")
                nc.vector.reciprocal(rstd[:], sd1[:])
                rstds.append((m, rstd))
            for qs in range(QS):
                m, rstd = rstds[qs]
                x = xs[qs]
                xc = ln_sb.tile([128, D], F32, tag="xc", name="xc")
                nc.vector.tensor_scalar(xc[:], x[:], m[:], rstd[:], ALU.subtract, ALU.mult)
                t2 = ln_sb.tile([128, D], F32, tag="t2", name="t2")
                nc.vector.tensor_tensor(t2[:], xc[:], ln_b["ln1_g"][:], ALU.mult)
                nc.vector.tensor_tensor(
                    z2_sb[:, qs * D:(qs + 1) * D], t2[:], ln_b["ln1_b"][:], ALU.add)
                # transpose z2 for expert/router matmuls
                for dd in range(2):
                    tp = tp2_ps.tile([128, 128], F32, tag="tp2", name="tp2")
                    nc.tensor.transpose(
                        tp[:], z2_sb[:, qs * D + dd * 128:qs * D + dd * 128 + 128], ident[:])
                    nc.vector.tensor_copy(z2T[dd][:, qs * 128:(qs + 1) * 128], tp[:])

            # router logits + softmax + top-1 gate
            lps_l, pr_l = [], []
            for qs in range(QS):
                lps = r_ps.tile([128, E], F32, tag="lps", name="lps")
                nc.tensor.matmul(lps[:], lhsT=z2T[0][:, qs * 128:(qs + 1) * 128],
                                 rhs=wr_a[:], start=True, stop=False)
                nc.tensor.matmul(lps[:], lhsT=z2T[1][:, qs * 128:(qs + 1) * 128],
                                 rhs=wr_b[:], start=False, stop=False)
                nc.tensor.matmul(lps[:], lhsT=grad_sb[0:1, qs * 128:(qs + 1) * 128],
                                 rhs=wr_c[:], start=False, stop=False)
                nc.tensor.matmul(lps[:], lhsT=ones1[:], rhs=brt_row[:],
                                 start=False, stop=True)
                nm = ln_sb.tile([128, 1], F32, tag=f"nm{qs}", name=f"nm{qs}")
                nc.vector.tensor_reduce(nm[:], lps[:], mybir.AxisListType.X, ALU.max,
                                        negate=True)
                lps_l.append(lps)
                pr_l.append(nm)
            for qs in range(QS):
                lps, nm = lps_l[qs], pr_l[qs]
                ex = ln_sb.tile([128, E], F32, tag=f"ex{qs}", name=f"ex{qs}")
                nc.scalar.activation(ex[:], lps[:], AF.Exp, bias=nm[:])
                se = ln_sb.tile([128, 1], F32, tag="se", name="se")
                nc.vector.tensor_reduce(se[:], ex[:], mybir.AxisListType.X, ALU.add)
                rse = ln_sb.tile([128, 1], F32, tag="rse", name="rse")
                nc.vector.reciprocal(rse[:], se[:])
                pr = ln_sb.tile([128, E], F32, tag="pr", name="pr")
                nc.vector.tensor_scalar(pr[:], ex[:], rse[:], None, ALU.mult)
                pm = ln_sb.tile([128, 1], F32, tag="pm", name="pm")
                nc.vector.tensor_reduce(pm[:], pr[:], mybir.AxisListType.X, ALU.max)
                mk = ln_sb.tile([128, E], F32, tag="mk", name="mk")
                nc.vector.tensor_scalar(mk[:], pr[:], pm[:], None, ALU.is_ge)
                nc.vector.tensor_tensor(
                    gate_sb[:, qs * E:(qs + 1) * E], pr[:], mk[:], ALU.mult)

    attnio_cm.__exit__(None, None, None)
    zT_cm.__exit__(None, None, None)

    # =====================================================================
    # Phase 4: MoE experts (dense compute, gated combine)
    # =====================================================================
    with pool("moe_w", 2) as moe_w, \
         pool("moe_h", 2) as moe_h, \
         pool("moe_sb", 2) as moe_sb, \
         pool("h_ps", 2, space="PSUM") as h_ps, \
         pool("y_ps", 2, space="PSUM") as y_ps:
        b2e_sb = moe_sb.tile([1, E * D], F32, tag="b2e", name="b2e", bufs=1)
        for e in range(E):
            dma(b2e_sb[:, e * D:(e + 1) * D], io["b2e"][e:e + 1, :])
        acc = [moe_sb.tile([128, D], F32, tag=f"acc{qs}", name=f"acc{qs}") for qs in range(QS)]
        for e in range(E):
            w1t = moe_w.tile([128, 2 * 4 * D], F32, tag="w1t", name="w1t")
            for kk in range(2):
                dma(w1t[:, kk * 4 * D:(kk + 1) * 4 * D],
                    io["W1e"][e, kk * 128:(kk + 1) * 128, :])
            w2t = moe_w.tile([128, 8 * D], F32, tag="w2t", name="w2t")
            for kk in range(8):
                dma(w2t[:, kk * D:(kk + 1) * D],
                    io["W2e"][e, kk * 128:(kk + 1) * 128, :])
            b1c = moe_w.tile([128, 8], F32, tag="b1c", name="b1c")
            dma(b1c[:], io["b1e"][e].rearrange("(f p) -> p f", p=128))

            hsb = moe_h.tile([128, 8 * S], F32, tag="hsb", name="hsb")
            for ft in range(8):
                hps = h_ps.tile([128, S], F32, tag="hps", name="hps")
                for kk in range(2):
                    nc.tensor.matmul(
                        hps[:],
                        lhsT=w1t[:, kk * 4 * D + ft * 128:kk * 4 * D + ft * 128 + 128],
                        rhs=z2T[kk][:], start=(kk == 0), stop=(kk == 1))
                nc.scalar.activation(hsb[:, ft * S:(ft + 1) * S], hps[:], AF.Gelu,
                                     bias=b1c[:, ft:ft + 1])
            for qs in range(QS):
                yps = y_ps.tile([128, D], F32, tag="yps", name="yps")
                for ft in range(8):
                    nc.tensor.matmul(
                        yps[:], lhsT=hsb[:, ft * S + qs * 128:ft * S + qs * 128 + 128],
                        rhs=w2t[:, ft * D:(ft + 1) * D], start=(ft == 0), stop=False)
                nc.tensor.matmul(yps[:], lhsT=ones1[:], rhs=b2e_sb[:, e * D:(e + 1) * D],
                                 start=False, stop=True)
                gt = moe_sb.tile([128, D], F32, tag="gt", name="gt")
                nc.scalar.mul(gt[:], yps[:], gate_sb[:, qs * E + e:qs * E + e + 1])
                if e == 0:
                    nc.vector.tensor_copy(acc[qs][:], gt[:])
                else:
                    nc.vector.tensor_tensor(acc[qs][:], acc[qs][:], gt[:], ALU.add)
        for qs in range(QS):
            nc.vector.tensor_tensor(z3_sb[:, qs * D:(qs + 1) * D],
                                    z2_sb[:, qs * D:(qs + 1) * D], acc[qs][:], ALU.add)

    # =====================================================================
    # Phase 5: gene decoder + functional head
    # =====================================================================
    with pool("dec_sb", 1) as dec_sb, \
         pool("dec_w", 3) as dec_w, \
         pool("out_sb", 4) as out_sb, \
         pool("d_ps", 1, space="PSUM") as d_ps, \
         pool("tp3_ps", 2, space="PSUM") as tp3_ps:
        bg2_row = dec_sb.tile([1, 2 * G], F32, tag="bg2r", name="bg2r")
        dma(bg2_row[:], io["bg2"].rearrange("(o x) -> o x", o=1))
        z3T = [dec_sb.tile([128, S], F32, tag=f"z3T{dd}", name=f"z3T{dd}") for dd in range(2)]
        dT = [dec_sb.tile([128, S], F32, tag=f"dT{dd}", name=f"dT{dd}") for dd in range(2)]
        wg1 = dec_sb.tile([128, 2 * D], F32, tag="wg1", name="wg1")
        for kk in range(2):
            dma(wg1[:, kk * D:(kk + 1) * D], io["Wg1"][kk * 128:(kk + 1) * 128, :])
        for qs in range(QS):
            for dd in range(2):
                tp = tp3_ps.tile([128, 128], F32, tag="tp3", name="tp3")
                nc.tensor.transpose(
                    tp[:], z3_sb[:, qs * D + dd * 128:qs * D + dd * 128 + 128], ident[:])
                nc.vector.tensor_copy(z3T[dd][:, qs * 128:(qs + 1) * 128], tp[:])

        # t = z3 @ Wg1 + bg1 ; LN ; gelu -> d (token-major then transposed)
        ts, stats = [], []
        for qs in range(QS):
            tps = d_ps.tile([128, D], F32, tag="tps", name="tps", bufs=2)
            for kk in range(2):
                nc.tensor.matmul(tps[:], lhsT=z3T[kk][:, qs * 128:(qs + 1) * 128],
                                 rhs=wg1[:, kk * D:(kk + 1) * D],
                                 start=(kk == 0), stop=False)
            nc.tensor.matmul(tps[:], lhsT=ones1[:], rhs=bg1_row[:],
                             start=False, stop=True)
            x = dec_sb.tile([128, D], F32, tag=f"dx{qs}", name=f"dx{qs}")
            sum_x = dec_sb.tile([128, 1], F32, tag=f"dsx{qs}", name=f"dsx{qs}")
            nc.scalar.activation(x[:], tps[:], AF.Copy, accum_out=sum_x[:])
            scratch = dec_sb.tile([128, D], F32, tag="dscr", name="dscr")
            sum_x2 = dec_sb.tile([128, 1], F32, tag=f"dsx2{qs}", name=f"dsx2{qs}")
            nc.scalar.activation(scratch[:], x[:], AF.Square, accum_out=sum_x2[:])
            ts.append(x)
            stats.append((sum_x, sum_x2))
        rstds = []
        for qs in range(QS):
            sum_x, sum_x2 = stats[qs]
            m = dec_sb.tile([128, 1], F32, tag=f"dm{qs}", name=f"dm{qs}")
            nc.vector.tensor_scalar(m[:], sum_x[:], 1.0 / D, None, ALU.mult)
            msq = dec_sb.tile([128, 1], F32, tag="dmsq", name="dmsq")
            nc.vector.tensor_tensor(msq[:], m[:], m[:], ALU.mult)
            var = dec_sb.tile([128, 1], F32, tag=f"dvar{qs}", name=f"dvar{qs}")
            nc.vector.tensor_scalar(var[:], sum_x2[:], 1.0 / D, msq[:], ALU.mult, ALU.subtract)
            sd = dec_sb.tile([128, 1], F32, tag=f"dsd{qs}", name=f"dsd{qs}")
            nc.scalar.activation(sd[:], var[:], AF.Sqrt, bias=eps5_c[:])
            veps = dec_sb.tile([128, 1], F32, tag="dveps", name="dveps")
            nc.vector.tensor_scalar(veps[:], var[:], 1e-5, None, ALU.add)
            rsd = dec_sb.tile([128, 1], F32, tag="drsd", name="drsd")
            nc.vector.reciprocal(rsd[:], sd[:])
            q1 = dec_sb.tile([128, 1], F32, tag="dq1", name="dq1")
            nc.vector.tensor_tensor(q1[:], veps[:], rsd[:], ALU.mult)
            sd1 = dec_sb.tile([128, 1], F32, tag=f"dsd1{qs}", name=f"dsd1{qs}")
            nc.vector.tensor_scalar(sd1[:], q1[:], sd[:], 0.5, ALU.add, ALU.mult)
            rstd = dec_sb.tile([128, 1], F32, tag=f"drstd{qs}", name=f"drstd{qs}")
            nc.vector.reciprocal(rstd[:], sd1[:])
            rstds.append((m, rstd))
        for qs in range(QS):
            m, rstd = rstds[qs]
            xc = dec_sb.tile([128, D], F32, tag="dxc", name="dxc")
            nc.vector.tensor_scalar(xc[:], ts[qs][:], m[:], rstd[:], ALU.subtract, ALU.mult)
            t2 = dec_sb.tile([128, D], F32, tag="dt2", name="dt2")
            nc.vector.tensor_tensor(t2[:], xc[:], ln_b["lng_g"][:], ALU.mult)
            t3 = dec_sb.tile([128, D], F32, tag="dt3", name="dt3")
            nc.vector.tensor_tensor(t3[:], t2[:], ln_b["lng_b"][:], ALU.add)
            dtok = dec_sb.tile([128, D], F32, tag="dtok", name="dtok")
            nc.scalar.activation(dtok[:], t3[:], AF.Gelu)
            for dd in range(2):
                tp = tp3_ps.tile([128, 128], F32, tag="tp3", name="tp3")
                nc.tensor.transpose(tp[:], dtok[:, dd * 128:(dd + 1) * 128], ident[:])
                nc.vector.tensor_copy(dT[dd][:, qs * 128:(qs + 1) * 128], tp[:])

        # functional head: g = sigmoid(gelu(z3 @ Wf1 + bf1) @ Wf2 + bf2)
        wf1 = dec_sb.tile([128, 2 * 64], F32, tag="wf1", name="wf1")
        for kk in range(2):
            dma(wf1[:, kk * 64:(kk + 1) * 64], io["Wf1"][kk * 128:(kk + 1) * 128, :])
        wf2 = dec_sb.tile([64, 1], F32, tag="wf2", name="wf2")
        dma(wf2[:], io["Wf2"][:])
        fps = d_ps.tile([64, S], F32, tag="fps", name="fps")
        for kk in range(2):
            nc.tensor.matmul(fps[:], lhsT=wf1[:, kk * 64:(kk + 1) * 64],
                             rhs=z3T[kk][:], start=(kk == 0), stop=(kk == 1))
        fg = dec_sb.tile([64, S], F32, tag="fg", name="fg")
        nc.scalar.activation(fg[:], fps[:], AF.Gelu, bias=bf1_c[:])
        for qs in range(QS):
            gps = d_ps.tile([128, 1], F32, tag="gps", name="gps", bufs=1)
            nc.tensor.matmul(gps[:], lhsT=fg[:, qs * 128:(qs + 1) * 128], rhs=wf2[:],
                             start=True, stop=False)
            nc.tensor.matmul(gps[:], lhsT=ones1[:], rhs=bf2_row[:],
                             start=False, stop=True)
            gsb = dec_sb.tile([128, 1], F32, tag="gsb", name="gsb")
            nc.scalar.activation(gsb[:], gps[:], AF.Sigmoid)
            dma(io["g_out"][qs * 128:(qs + 1) * 128, :], gsb[:])

        # preds = d @ Wg2 + bg2 ; softplus ; (+1e-6 on theta columns)
        for ft in range(8):
            f0 = ft * 512
            fw = min(512, 2 * G - f0)
            wg2t = dec_w.tile([128, 2 * 512], F32, tag="wg2t", name="wg2t")
            for kk in range(2):
                dma(wg2t[:, kk * 512:kk * 512 + fw],
                    io["Wg2"][kk * 128:(kk + 1) * 128, f0:f0 + fw])
            for qs in range(QS):
                pps = d_ps.tile([128, 512], F32, tag="pps", name="pps", bufs=2)
                for kk in range(2):
                    nc.tensor.matmul(pps[:, 0:fw], lhsT=dT[kk][:, qs * 128:(qs + 1) * 128],
                                     rhs=wg2t[:, kk * 512:kk * 512 + fw],
                                     start=(kk == 0), stop=False)
                nc.tensor.matmul(pps[:, 0:fw], lhsT=ones1[:], rhs=bg2_row[0:1, f0:f0 + fw],
                                 start=False, stop=True)
                esb = out_sb.tile([128, 512], F32, tag="esb", name="esb")
                nc.scalar.activation(esb[:, 0:fw], pps[:, 0:fw], AF.Exp)
                osb = out_sb.tile([128, 512], F32, tag="osb", name="osb")
                nc.scalar.activation(osb[:, 0:fw], esb[:, 0:fw], AF.Ln, bias=1.0)
                nc.vector.tensor_tensor(osb[:, 0:fw], osb[:, 0:fw], eps_t[:, 0:fw], ALU.add)
                dma(io["preds_out"][qs * 128:(qs + 1) * 128, f0:f0 + fw], osb[:, 0:fw])

    es.close()


def build_program():
    nc = bacc.Bacc("TRN2", target_bir_lowering=False, debug=False,
                   num_devices=NCORES)
    io = {}

    def inp(name, shape):
        io[name] = nc.dram_tensor(name, list(shape), F32, kind="ExternalInput").ap()

    inp("visT", [DU, N])
    inp("posT", [3, N])
    inp("gradT", [1, S])
    inp("B_fourier", [3, MAP])
    inp("W_img", [DU, D]); inp("b_img", [D])
    inp("W_pos", [2 * MAP, D]); inp("b_pos", [D])
    for w in ("Wq", "Wk", "Wv", "Wo"):
        inp(w, [D, D])
    for b in ("bq", "bk", "bv", "bo"):
        inp(b, [D])
    inp("ln1_g", [D]); inp("ln1_b", [D])
    inp("W_router", [D + 1, E]); inp("b_router", [E])
    inp("W1e", [E, D, 4 * D]); inp("b1e", [E, 4 * D])
    inp("W2e", [E, 4 * D, D]); inp("b2e", [E, D])
    inp("Wg1", [D, D]); inp("bg1", [D])
    inp("lng_g", [D]); inp("lng_b", [D])
    inp("Wg2", [D, 2 * G]); inp("bg2", [2 * G])
    inp("Wf1", [D, 64]); inp("bf1", [64])
    inp("Wf2", [64, 1]); inp("bf2", [1])
    io["preds_out"] = nc.dram_tensor("preds_out", [S, 2 * G], F32,
                                     kind="ExternalOutput").ap()
    io["g_out"] = nc.dram_tensor("g_out", [S, 1], F32, kind="ExternalOutput").ap()

    with tile.TileContext(nc) as tc:
        _build_body(nc, tc, io)
    nc.compile()
    return nc


_PROGRAM = None


def get_program():
    global _PROGRAM
    if _PROGRAM is None:
        _PROGRAM = build_program()
    return _PROGRAM


def prep_inputs(inputs):
    """Build the 8 per-core input maps (token-rotated full inputs)."""
    f = lambda k: np.ascontiguousarray(np.asarray(inputs[k], dtype=np.float32))
    vis_T = f("vis").T.copy()          # [1024, 4096]
    pos_T = f("pos").T.copy()          # [3, 4096]
    grad = f("grad")                   # [4096, 1]
    shared = {}
    for k in ("B_fourier", "W_img", "b_img", "W_pos", "b_pos", "Wq", "bq", "Wk",
              "bk", "Wv", "bv", "Wo", "bo", "ln1_g", "ln1_b", "W_router",
              "b_router", "W1e", "b1e", "W2e", "b2e", "Wg1", "bg1", "lng_g",
              "lng_b", "Wg2", "bg2", "Wf1", "bf1", "Wf2", "bf2"):
        shared[k] = f(k)
    in_maps = []
    for c in range(NCORES):
        o = c * S
        m = dict(shared)
        m["visT"] = np.ascontiguousarray(np.roll(vis_T, -o, axis=1))
        m["posT"] = np.ascontiguousarray(np.roll(pos_T, -o, axis=1))
        m["gradT"] = np.ascontiguousarray(grad[o:o + S, 0][None, :])
        in_maps.append(m)
    return in_maps


def kernel(**inputs):
    nc = get_program()
    in_maps = prep_inputs(inputs)
    res = run_bass_kernel_spmd(nc, in_maps, core_ids=list(range(NCORES)))
    preds = np.concatenate([res.results[c]["preds_out"] for c in range(NCORES)], 0)
    g = np.concatenate([res.results[c]["g_out"] for c in range(NCORES)], 0)
    preds = preds.reshape(N, G, 2)
    mu = np.ascontiguousarray(preds[:, :, 0])
    theta = np.ascontiguousarray(preds[:, :, 1])
    return mu, theta, g


# revision 28
# speedup vs baseline: 1.8420x; 1.8420x over previous
"""Trainium2 Bass kernel for nn_MoEST_Plus (MoE spatial transformer).

Sharding: data-parallel over the N (spot) axis. Each of the 8 cores
receives a token-rotated copy of the full inputs so that its OWN 512
tokens sit at positions 0..511; the encoder (z) is computed for all 4096
tokens on every core (attention needs full K/V; replication avoids
collectives), while attention-queries / MoE / decoders run only on the
core's own 512 tokens. Outputs are gathered and de-interleaved on host.

Precision: heavy matmuls run in float32r (1 cycle/row vs 4 for fp32).
The router-feeding path (z for the core's own tokens -> LN1 -> router
logits) is kept in full fp32 so top-1 expert selection matches the
reference even for near-tied router probabilities.
"""

import os
import sys

import numpy as np

for _p in ("/opt/trn_rl_repo",):
    if os.path.isdir(_p) and _p not in sys.path:
        sys.path.insert(0, _p)

import concourse.bacc as bacc
import concourse.mybir as mybir
import concourse.tile as tile
from concourse import masks
from concourse.bass_utils import run_bass_kernel_spmd

AF = mybir.ActivationFunctionType
ALU = mybir.AluOpType
F32 = mybir.dt.float32
F32R = mybir.dt.float32r

N = 4096          # tokens (spots)
DU = 1024         # dim_uni
D = 256           # dim_hidden
G = 2000          # genes
E = 4             # experts
H = 4             # heads
HD = 64           # head dim
MAP = 64          # fourier mapping size
NCORES = 8
S = N // NCORES   # own tokens per core = 512
QS = S // 128     # 128-token q tiles = 4
TT = N // 512     # 512-token tiles = 8
KT = N // 128     # 128-token key tiles = 32
PI = float(np.pi)
MAGIC = 12582912.0  # 1.5 * 2**23 fp32 round-to-nearest constant


def _build_body(nc, tc, io):
    from contextlib import ExitStack

    es = ExitStack()

    def pool(name, bufs, space="SBUF"):
        return tc.tile_pool(name=name, bufs=bufs, space=space)

    def dma(dst, src):
        nc.sync.dma_start(dst, src)

    def dmar(dst, src):
        """DMA fp32 DRAM -> f32r SBUF tile (hardware rounds)."""
        nc.sync.dma_start(dst, src.bitcast(F32R))

    # ---------------- constants / small params ----------------
    cpool = es.enter_context(tc.tile_pool(name="consts", bufs=1))
    ident = cpool.tile([128, 128], F32, tag="ident", name="ident")
    masks.make_identity(nc, ident[:])
    ones1 = cpool.tile([1, 128], F32, tag="ones1", name="ones1")
    nc.gpsimd.memset(ones1[:], 1.0)
    ones_b = cpool.tile([128, D], F32, tag="ones_b", name="ones_b")
    nc.gpsimd.memset(ones_b[:], 1.0)
    eps5_c = cpool.tile([128, 1], F32, tag="eps5_c", name="eps5_c")
    nc.gpsimd.memset(eps5_c[:], 1e-5)

    def row(dram_1d, n, tag):
        t = cpool.tile([1, n], F32, tag=tag, name=tag)
        dma(t[:], dram_1d.rearrange("(o x) -> o x", o=1))
        return t

    def col(dram_1d, lo, n, tag):
        t = cpool.tile([n, 1], F32, tag=tag, name=tag)
        dma(t[:], dram_1d[lo:lo + n].rearrange("(p o) -> p o", o=1))
        return t

    bimg_c = [col(io["b_img"], dd * 128, 128, f"bimg{dd}") for dd in range(2)]
    bpos_c = [col(io["b_pos"], dd * 128, 128, f"bpos{dd}") for dd in range(2)]
    bz_c = []
    for dd in range(2):
        t = cpool.tile([128, 1], F32, tag=f"bz{dd}", name=f"bz{dd}")
        nc.vector.tensor_add(t[:], bimg_c[dd][:], bpos_c[dd][:])
        bz_c.append(t)
    bk_c = [col(io["bk"], dd * 128, 128, f"bk{dd}") for dd in range(2)]
    bq_c = [col(io["bq"], dd * 128, 128, f"bq{dd}") for dd in range(2)]
    bf1_c = col(io["bf1"], 0, 64, "bf1")

    bv_row = row(io["bv"], D, "bvr")
    bo_row = row(io["bo"], D, "bor")
    brt_row = row(io["b_router"], E, "brtr")
    bg1_row = row(io["bg1"], D, "bg1r")
    bf2_row = row(io["bf2"], 1, "bf2r")

    # LN gamma/beta broadcast tiles [128, 256] via PE outer-product
    ln_rows = {k: row(io[k], D, k + "r") for k in ("ln1_g", "ln1_b", "lng_g", "lng_b")}
    ln_b = {}
    with pool("lnb_ps", 2, space="PSUM") as lnps:
        for k, r in ln_rows.items():
            ps = lnps.tile([128, D], F32, tag="lnb", name="lnb")
            nc.tensor.matmul(ps[:], lhsT=ones1[:], rhs=r[:], start=True, stop=True)
            t = cpool.tile([128, D], F32, tag=k + "b", name=k + "b")
            nc.vector.tensor_copy(t[:], ps[:])
            ln_b[k] = t

    # eps tile for theta += 1e-6 (odd output columns)
    eps_t = cpool.tile([128, 1024], F32, tag="epst", name="epst")
    nc.gpsimd.memset(eps_t[:], 0.0)
    nc.gpsimd.memset(eps_t[:].rearrange("p (g two) -> p g two", two=2)[:, :, 1:2], 1e-6)

    bfour = cpool.tile([3, MAP], F32, tag="bfour", name="bfour")
    dma(bfour[:], io["B_fourier"][:])

    # ---------------- persistent activation tiles ----------------
    act_pool = es.enter_context(tc.tile_pool(name="acts", bufs=1))
    z_own = act_pool.tile([128, QS * D], F32, tag="z_own", name="z_own")
    z2_sb = act_pool.tile([128, QS * D], F32, tag="z2", name="z2")
    z2T = [act_pool.tile([128, S], F32, tag=f"z2T{dd}", name=f"z2T{dd}") for dd in range(2)]
    z2Tr = [act_pool.tile([128, S], F32R, tag=f"z2Tr{dd}", name=f"z2Tr{dd}") for dd in range(2)]
    z3_sb = act_pool.tile([128, QS * D], F32, tag="z3", name="z3")
    gate_sb = act_pool.tile([128, QS * E], F32, tag="gate", name="gate")

    # =====================================================================
    # Phase 1+2: encoder (z_T full f32r + fp32 own slice) then k/v/q
    # =====================================================================
    zT_cm = tc.tile_pool(name="zT_pool", bufs=1)
    zT_pool = zT_cm.__enter__()
    zT = [zT_pool.tile([128, N], F32R, tag=f"zT{dd}", name=f"zT{dd}") for dd in range(2)]
    z_ownT = [zT_pool.tile([128, S], F32, tag=f"z_ownT{dd}", name=f"z_ownT{dd}")
              for dd in range(2)]

    encw_cm = tc.tile_pool(name="enc_sb", bufs=1)
    enc_sb = encw_cm.__enter__()
    with pool("vis_sb", 6) as vis_pool, \
         pool("four_sb", 2) as four_pool, \
         pool("enc_ps", 4, space="PSUM") as enc_ps, \
         pool("fours_ps", 2, space="PSUM") as four_ps:
        wimg_r = enc_sb.tile([128, 8 * D], F32R, tag="wimg_r", name="wimg_r")
        wimg = enc_sb.tile([128, 8 * D], F32, tag="wimg", name="wimg")
        for kk in range(8):
            dmar(wimg_r[:, kk * D:(kk + 1) * D], io["W_img"][kk * 128:(kk + 1) * 128, :])
            dma(wimg[:, kk * D:(kk + 1) * D], io["W_img"][kk * 128:(kk + 1) * 128, :])
        wpos_r = enc_sb.tile([128, D], F32R, tag="wpos_r", name="wpos_r")
        dmar(wpos_r[:], io["W_pos"][:])
        wpos = enc_sb.tile([128, D], F32, tag="wpos", name="wpos")
        dma(wpos[:], io["W_pos"][:])

        for tt in range(TT):
            t0 = tt * 512
            # fourier features (fp32, range-reduced)
            pos_sb = four_pool.tile([3, 512], F32, tag="pos", name="pos")
            dma(pos_sb[:], io["posT"][:, t0:t0 + 512])
            xp = four_ps.tile([MAP, 512], F32, tag="xp", name="xp")
            nc.tensor.matmul(xp[:], lhsT=bfour[:], rhs=pos_sb[:], start=True, stop=True)
            r1 = four_pool.tile([MAP, 512], F32, tag="r1", name="r1")
            nc.vector.tensor_scalar(r1[:], xp[:], MAGIC, -MAGIC, ALU.add, ALU.add)
            fx = four_pool.tile([128, 512], F32, tag="fx", name="fx")
            nc.vector.tensor_tensor(fx[0:MAP, :], xp[:], r1[:], ALU.subtract)
            t2 = four_pool.tile([MAP, 512], F32, tag="t2", name="t2")
            nc.vector.tensor_scalar(t2[:], xp[:], 0.25, None, ALU.add)
            r2 = four_pool.tile([MAP, 512], F32, tag="r2", name="r2")
            nc.vector.tensor_scalar(r2[:], t2[:], MAGIC, -MAGIC, ALU.add, ALU.add)
            nc.vector.tensor_tensor(fx[MAP:128, :], t2[:], r2[:], ALU.subtract)
            four = four_pool.tile([128, 512], F32, tag="four", name="four")
            nc.scalar.activation(four[:], fx[:], AF.Sin, scale=2 * PI)
            four_r = four_pool.tile([128, 512], F32R, tag="four_rr", name="four_rr")
            nc.vector.tensor_copy(four_r[:], four[:])

            vtiles = []
            for kk in range(8):
                vt = vis_pool.tile([128, 512], F32R, tag="vis", name="vis")
                dmar(vt[:], io["visT"][kk * 128:(kk + 1) * 128, t0:t0 + 512])
                vtiles.append(vt)

            for dd in range(2):
                zps = enc_ps.tile([128, 512], F32, tag="zps", name="zps")
                for kk in range(8):
                    nc.tensor.matmul(
                        zps[:], lhsT=wimg_r[:, kk * D + dd * 128:kk * D + dd * 128 + 128],
                        rhs=vtiles[kk][:], start=(kk == 0), stop=False)
                nc.tensor.matmul(
                    zps[:], lhsT=wpos_r[:, dd * 128:dd * 128 + 128],
                    rhs=four_r[:], start=False, stop=True)
                nc.vector.tensor_scalar(zT[dd][:, t0:t0 + 512], zps[:], bz_c[dd][:], None, ALU.add)

            if tt == 0:
                # fp32 own-slice z (feeds LN1 -> router; keeps routing faithful)
                votiles = []
                for kk in range(8):
                    vo = vis_pool.tile([128, 512], F32, tag="vis_o", name="vis_o")
                    dma(vo[:], io["visT"][kk * 128:(kk + 1) * 128, 0:512])
                    votiles.append(vo)
                for dd in range(2):
                    zops = enc_ps.tile([128, 512], F32, tag="zops", name="zops", bufs=2)
                    for kk in range(8):
                        nc.tensor.matmul(
                            zops[:], lhsT=wimg[:, kk * D + dd * 128:kk * D + dd * 128 + 128],
                            rhs=votiles[kk][:], start=(kk == 0), stop=False)
                    nc.tensor.matmul(
                        zops[:], lhsT=wpos[:, dd * 128:dd * 128 + 128],
                        rhs=four[:], start=False, stop=True)
                    nc.vector.tensor_scalar(z_ownT[dd][:], zops[:], bz_c[dd][:], None, ALU.add)

    encw_cm.__exit__(None, None, None)

    attnio_cm = tc.tile_pool(name="attn_io", bufs=1)
    attn_io = attnio_cm.__enter__()
    kT = [attn_io.tile([128, N], F32R, tag=f"kT{dd}", name=f"kT{dd}") for dd in range(2)]
    v_sb = attn_io.tile([128, KT * (H * (HD + 1))], F32R, tag="v1", name="v1")
    qT = [attn_io.tile([128, S], F32R, tag=f"qT{dd}", name=f"qT{dd}") for dd in range(2)]
    _vones = v_sb[:].rearrange("p (t c) -> p t c", c=HD + 1)[:, :, HD]
    nc.vector.tensor_copy(_vones, ones_b[:, 0:KT * H])

    if True:
        with pool("kvq_sb", 1) as kvq_sb, \
             pool("kvq_ps", 3, space="PSUM") as kvq_ps, \
             pool("tp_ps", 2, space="PSUM") as tp_ps:
            wk = kvq_sb.tile([128, 2 * D], F32R, tag="wk", name="wk")
            wq = kvq_sb.tile([128, 2 * D], F32R, tag="wq", name="wq")
            wv = kvq_sb.tile([128, 2 * D], F32R, tag="wv", name="wv")
            for kk in range(2):
                dmar(wk[:, kk * D:(kk + 1) * D], io["Wk"][kk * 128:(kk + 1) * 128, :])
                dmar(wq[:, kk * D:(kk + 1) * D], io["Wq"][kk * 128:(kk + 1) * 128, :])
                dmar(wv[:, kk * D:(kk + 1) * D], io["Wv"][kk * 128:(kk + 1) * 128, :])

            # k_T (feature-major, all tokens)
            for dd in range(2):
                for tt in range(TT):
                    t0 = tt * 512
                    kps = kvq_ps.tile([128, 512], F32, tag="kps", name="kps")
                    for kk in range(2):
                        nc.tensor.matmul(
                            kps[:], lhsT=wk[:, kk * D + dd * 128:kk * D + dd * 128 + 128],
                            rhs=zT[kk][:, t0:t0 + 512], start=(kk == 0), stop=(kk == 1))
                    nc.vector.tensor_scalar(kT[dd][:, t0:t0 + 512], kps[:], bk_c[dd][:], None, ALU.add)

            # q_T (feature-major, own tokens)
            for dd in range(2):
                qps = kvq_ps.tile([128, 512], F32, tag="kps", name="qps")
                for kk in range(2):
                    nc.tensor.matmul(
                        qps[:], lhsT=wq[:, kk * D + dd * 128:kk * D + dd * 128 + 128],
                        rhs=zT[kk][:, 0:512], start=(kk == 0), stop=(kk == 1))
                nc.vector.tensor_scalar(qT[dd][:], qps[:], bq_c[dd][:], None, ALU.add)

            # v (token-major, all tokens) with bias via ones-row
            for kt in range(KT):
                p0 = kt * 128
                vps = kvq_ps.tile([128, D], F32, tag="vps", name="vps")
                for kk in range(2):
                    nc.tensor.matmul(
                        vps[:], lhsT=zT[kk][:, p0:p0 + 128],
                        rhs=wv[:, kk * D:(kk + 1) * D], start=(kk == 0), stop=False)
                nc.tensor.matmul(vps[:], lhsT=ones1[:], rhs=bv_row[:],
                                 start=False, stop=True)
                dst = v_sb[:, kt * H * (HD + 1):(kt + 1) * H * (HD + 1)]
                dst = dst.rearrange("p (h c) -> p h c", c=HD + 1)[:, :, 0:HD]
                nc.vector.tensor_copy(dst, vps[:].rearrange("p (h c) -> p h c", c=HD))

            # z_own (token-major, fp32) from fp32 z_ownT
            for qs in range(QS):
                for dd in range(2):
                    tp = tp_ps.tile([128, 128], F32, tag="tp", name="tp")
                    nc.tensor.transpose(tp[:], z_ownT[dd][:, qs * 128:(qs + 1) * 128],
                                        ident[:])
                    nc.vector.tensor_copy(
                        z_own[:, qs * D + dd * 128:qs * D + dd * 128 + 128], tp[:])

    # =====================================================================
    # Phase 3: attention (own 512 queries over all 4096 keys)
    # =====================================================================
    with pool("attn_sb", 1) as attn_sb, \
         pool("p_sb", 4) as p_pool:
        wo_sb = attn_sb.tile([HD, H * D], F32R, tag="wo", name="wo")
        for h in range(H):
            dmar(wo_sb[:, h * D:(h + 1) * D], io["Wo"][h * HD:(h + 1) * HD, :])
        grad_sb = attn_sb.tile([1, S], F32, tag="grad", name="grad")
        dma(grad_sb[:], io["gradT"][:])
        wr_a = attn_sb.tile([128, E], F32, tag="wra", name="wra")
        wr_b = attn_sb.tile([128, E], F32, tag="wrb", name="wrb")
        wr_c = attn_sb.tile([1, E], F32, tag="wrc", name="wrc")
        dma(wr_a[:], io["W_router"][0:128, :])
        dma(wr_b[:], io["W_router"][128:256, :])
        dma(wr_c[:], io["W_router"][256:257, :])
        ctxn = [attn_sb.tile([HD, S], F32R, tag=f"ctxn{h}", name=f"ctxn{h}")
                for h in range(H)]
        heads_ps_cm = [pool("s_ps", 2, space="PSUM"), pool("ctx_ps", 2, space="PSUM")]
        s_ps = heads_ps_cm[0].__enter__()
        ctx_ps_pool = heads_ps_cm[1].__enter__()
        for h in range(H):
            dd, off = h // 2, (h % 2) * HD
            ctx_ps = ctx_ps_pool.tile([HD + 1, S], F32, tag="ctx", name="ctx")
            for kp in range(KT // 2):
                sps = s_ps.tile([128, 2 * S], F32, tag="sps", name="sps")
                for half in range(2):
                    kt = 2 * kp + half
                    nc.tensor.matmul(
                        sps[:, half * S:(half + 1) * S],
                        lhsT=kT[dd][off:off + HD, kt * 128:(kt + 1) * 128],
                        rhs=qT[dd][off:off + HD, :], start=True, stop=True)
                pt = p_pool.tile([128, 2 * S], F32R, tag="pt", name="pt")
                nc.scalar.activation(pt[:], sps[:], AF.Exp, scale=1.0 / 8.0)
                for half in range(2):
                    kt = 2 * kp + half
                    c0 = kt * H * (HD + 1) + h * (HD + 1)
                    nc.tensor.matmul(ctx_ps[:], lhsT=v_sb[:, c0:c0 + HD + 1],
                                     rhs=pt[:, half * S:(half + 1) * S],
                                     start=(kt == 0), stop=(kt == KT - 1))
            rd = attn_sb.tile([1, S], F32, tag="rd", name="rd")
            nc.vector.reciprocal(rd[:], ctx_ps[HD:HD + 1, :])
            rb_ps = s_ps.tile([HD, S], F32, tag="rbps", name="rbps", bufs=1)
            nc.tensor.matmul(rb_ps[:], lhsT=ones1[0:1, 0:HD], rhs=rd[:],
                             start=True, stop=True)
            rb = attn_sb.tile([HD, S], F32, tag="rb", name="rb")
            nc.vector.tensor_copy(rb[:], rb_ps[:])
            nc.vector.tensor_tensor(ctxn[h][:], ctx_ps[0:HD, :], rb[:], ALU.mult)

        heads_ps_cm[1].__exit__(None, None, None)
        heads_ps_cm[0].__exit__(None, None, None)

        # attn_out + residual + LN1 + router + gates
        with pool("wo_ps", 2, space="PSUM") as wo_ps, \
             pool("ln_sb", 1) as ln_sb, \
             pool("r_ps", 2, space="PSUM") as r_ps, \
             pool("tp2_ps", 2, space="PSUM") as tp2_ps:
            xs, stats = [], []
            for qs in range(QS):
                aps = wo_ps.tile([128, D], F32, tag="aps", name="aps")
                for h in range(H):
                    nc.tensor.matmul(
                        aps[:], lhsT=ctxn[h][:, qs * 128:(qs + 1) * 128],
                        rhs=wo_sb[:, h * D:(h + 1) * D],
                        start=(h == 0), stop=False)
                nc.tensor.matmul(aps[:], lhsT=ones1[:], rhs=bo_row[:],
                                 start=False, stop=True)
                x = ln_sb.tile([128, D], F32, tag=f"x{qs}", name=f"x{qs}")
                nc.vector.tensor_tensor(x[:], z_own[:, qs * D:(qs + 1) * D], aps[:],
                                        ALU.add)
                sum_x = ln_sb.tile([128, 1], F32, tag=f"sx{qs}", name=f"sx{qs}")
                scratch = ln_sb.tile([128, D], F32, tag="lnscr", name="lnscr")
                nc.scalar.activation(scratch[:], x[:], AF.Copy, accum_out=sum_x[:])
                sum_x2 = ln_sb.tile([128, 1], F32, tag=f"sx2{qs}", name=f"sx2{qs}")
                nc.scalar.activation(scratch[:], x[:], AF.Square, accum_out=sum_x2[:])
                xs.append(x)
                stats.append((sum_x, sum_x2))
            rstds = []
            for qs in range(QS):
                sum_x, sum_x2 = stats[qs]
                m = ln_sb.tile([128, 1], F32, tag=f"m{qs}", name=f"m{qs}")
                nc.vector.tensor_scalar(m[:], sum_x[:], 1.0 / D, None, ALU.mult)
                msq = ln_sb.tile([128, 1], F32, tag=f"msq{qs}", name=f"msq{qs}")
                nc.vector.tensor_tensor(msq[:], m[:], m[:], ALU.mult)
                var = ln_sb.tile([128, 1], F32, tag=f"var{qs}", name=f"var{qs}")
                nc.vector.tensor_scalar(var[:], sum_x2[:], 1.0 / D, msq[:],
                                        ALU.mult, ALU.subtract)
                lv = ln_sb.tile([128, 1], F32, tag=f"lv{qs}", name=f"lv{qs}")
                nc.scalar.activation(lv[:], var[:], AF.Ln, bias=eps5_c[:])
                rstd = ln_sb.tile([128, 1], F32, tag=f"rstd{qs}", name=f"rstd{qs}")
                nc.scalar.activation(rstd[:], lv[:], AF.Exp, scale=-0.5)
                rstds.append((m, rstd))
            for qs in range(QS):
                m, rstd = rstds[qs]
                x = xs[qs]
                xc = ln_sb.tile([128, D], F32, tag="xc", name="xc")
                nc.vector.tensor_scalar(xc[:], x[:], m[:], rstd[:],
                                        ALU.subtract, ALU.mult)
                t2 = ln_sb.tile([128, D], F32, tag="t2l", name="t2l")
                nc.vector.tensor_tensor(t2[:], xc[:], ln_b["ln1_g"][:], ALU.mult)
                nc.vector.tensor_tensor(
                    z2_sb[:, qs * D:(qs + 1) * D], t2[:], ln_b["ln1_b"][:], ALU.add)
                for dd in range(2):
                    tp = tp2_ps.tile([128, 128], F32, tag="tp2", name="tp2")
                    nc.tensor.transpose(
                        tp[:], z2_sb[:, qs * D + dd * 128:qs * D + dd * 128 + 128],
                        ident[:])
                    nc.vector.tensor_copy(z2T[dd][:, qs * 128:(qs + 1) * 128], tp[:])
                    nc.vector.tensor_copy(z2Tr[dd][:, qs * 128:(qs + 1) * 128], tp[:])

            # router logits + softmax + top-1 gate (fp32)
            lps_l, pr_l = [], []
            for qs in range(QS):
                lps = r_ps.tile([128, E], F32, tag="lps", name="lps")
                nc.tensor.matmul(lps[:], lhsT=z2T[0][:, qs * 128:(qs + 1) * 128],
                                 rhs=wr_a[:], start=True, stop=False)
                nc.tensor.matmul(lps[:], lhsT=z2T[1][:, qs * 128:(qs + 1) * 128],
                                 rhs=wr_b[:], start=False, stop=False)
                nc.tensor.matmul(lps[:], lhsT=grad_sb[0:1, qs * 128:(qs + 1) * 128],
                                 rhs=wr_c[:], start=False, stop=False)
                nc.tensor.matmul(lps[:], lhsT=ones1[:], rhs=brt_row[:],
                                 start=False, stop=True)
                nm = ln_sb.tile([128, 1], F32, tag=f"nm{qs}", name=f"nm{qs}")
                nc.vector.tensor_reduce(nm[:], lps[:], mybir.AxisListType.X, ALU.max,
                                        negate=True)
                lps_l.append(lps)
                pr_l.append(nm)
            for qs in range(QS):
                lps, nm = lps_l[qs], pr_l[qs]
                ex = ln_sb.tile([128, E], F32, tag=f"ex{qs}", name=f"ex{qs}")
                nc.scalar.activation(ex[:], lps[:], AF.Exp, bias=nm[:])
                se = ln_sb.tile([128, 1], F32, tag="se", name="se")
                nc.vector.tensor_reduce(se[:], ex[:], mybir.AxisListType.X, ALU.add)
                rse = ln_sb.tile([128, 1], F32, tag="rse", name="rse")
                nc.vector.reciprocal(rse[:], se[:])
                pr = ln_sb.tile([128, E], F32, tag="pr", name="pr")
                nc.vector.tensor_scalar(pr[:], ex[:], rse[:], None, ALU.mult)
                pm = ln_sb.tile([128, 1], F32, tag="pm", name="pm")
                nc.vector.tensor_reduce(pm[:], pr[:], mybir.AxisListType.X, ALU.max)
                mk = ln_sb.tile([128, E], F32, tag="mk", name="mk")
                nc.vector.tensor_scalar(mk[:], pr[:], pm[:], None, ALU.is_ge)
                nc.vector.tensor_tensor(
                    gate_sb[:, qs * E:(qs + 1) * E], pr[:], mk[:], ALU.mult)

    attnio_cm.__exit__(None, None, None)
    zT_cm.__exit__(None, None, None)

    # =====================================================================
    # Phase 4: MoE experts (dense f32r compute, gated combine)
    # =====================================================================
    with pool("moe_w", 2) as moe_w, \
         pool("moe_h", 2) as moe_h, \
         pool("moe_sb", 2) as moe_sb, \
         pool("h_ps", 2, space="PSUM") as h_ps, \
         pool("y_ps", 2, space="PSUM") as y_ps:
        b2e_sb = moe_sb.tile([1, E * D], F32, tag="b2e", name="b2e", bufs=1)
        for e in range(E):
            dma(b2e_sb[:, e * D:(e + 1) * D], io["b2e"][e:e + 1, :])
        acc = [moe_sb.tile([128, D], F32, tag=f"acc{qs}", name=f"acc{qs}")
               for qs in range(QS)]
        for e in range(E):
            w1t = moe_w.tile([128, 2 * 4 * D], F32R, tag="w1t", name="w1t")
            for kk in range(2):
                dmar(w1t[:, kk * 4 * D:(kk + 1) * 4 * D],
                     io["W1e"][e, kk * 128:(kk + 1) * 128, :])
            w2t = moe_w.tile([128, 8 * D], F32R, tag="w2t", name="w2t")
            for kk in range(8):
                dmar(w2t[:, kk * D:(kk + 1) * D],
                     io["W2e"][e, kk * 128:(kk + 1) * 128, :])
            b1c = moe_w.tile([128, 8], F32, tag="b1c", name="b1c")
            dma(b1c[:], io["b1e"][e].rearrange("(f p) -> p f", p=128))

            hsb = moe_h.tile([128, 8 * S], F32R, tag="hsb", name="hsb")
            for ft in range(8):
                hps = h_ps.tile([128, S], F32, tag="hps", name="hps")
                for kk in range(2):
                    nc.tensor.matmul(
                        hps[:],
                        lhsT=w1t[:, kk * 4 * D + ft * 128:kk * 4 * D + ft * 128 + 128],
                        rhs=z2Tr[kk][:], start=(kk == 0), stop=(kk == 1))
                nc.scalar.activation(hsb[:, ft * S:(ft + 1) * S], hps[:], AF.Gelu,
                                     bias=b1c[:, ft:ft + 1])
            for qs in range(QS):
                yps = y_ps.tile([128, D], F32, tag="yps", name="yps")
                for ft in range(8):
                    nc.tensor.matmul(
                        yps[:], lhsT=hsb[:, ft * S + qs * 128:ft * S + qs * 128 + 128],
                        rhs=w2t[:, ft * D:(ft + 1) * D], start=(ft == 0), stop=False)
                nc.tensor.matmul(yps[:], lhsT=ones1[:],
                                 rhs=b2e_sb[:, e * D:(e + 1) * D],
                                 start=False, stop=True)
                gt = moe_sb.tile([128, D], F32, tag="gt", name="gt")
                nc.vector.tensor_scalar(gt[:], yps[:],
                                        gate_sb[:, qs * E + e:qs * E + e + 1],
                                        None, ALU.mult)
                if e == 0:
                    nc.vector.tensor_copy(acc[qs][:], gt[:])
                else:
                    nc.vector.tensor_tensor(acc[qs][:], acc[qs][:], gt[:], ALU.add)
        for qs in range(QS):
            nc.vector.tensor_tensor(z3_sb[:, qs * D:(qs + 1) * D],
                                    z2_sb[:, qs * D:(qs + 1) * D], acc[qs][:], ALU.add)

    # =====================================================================
    # Phase 5: gene decoder + functional head
    # =====================================================================
    with pool("dec_sb", 1) as dec_sb, \
         pool("dec_w", 3) as dec_w, \
         pool("out_sb", 4) as out_sb:
        dps_cm = pool("d_ps", 1, space="PSUM")
        tp3_cm = pool("tp3_ps", 2, space="PSUM")
        d_ps = dps_cm.__enter__()
        tp3_ps = tp3_cm.__enter__()
        bg2_row = dec_sb.tile([1, 2 * G], F32, tag="bg2r", name="bg2r")
        dma(bg2_row[:], io["bg2"].rearrange("(o x) -> o x", o=1))
        z3T = [dec_sb.tile([128, S], F32R, tag=f"z3T{dd}", name=f"z3T{dd}")
               for dd in range(2)]
        dT = [dec_sb.tile([128, S], F32R, tag=f"dT{dd}", name=f"dT{dd}")
              for dd in range(2)]
        wg1 = dec_sb.tile([128, 2 * D], F32R, tag="wg1", name="wg1")
        for kk in range(2):
            dmar(wg1[:, kk * D:(kk + 1) * D], io["Wg1"][kk * 128:(kk + 1) * 128, :])
        for qs in range(QS):
            for dd in range(2):
                tp = tp3_ps.tile([128, 128], F32, tag="tp3", name="tp3")
                nc.tensor.transpose(
                    tp[:], z3_sb[:, qs * D + dd * 128:qs * D + dd * 128 + 128], ident[:])
                nc.vector.tensor_copy(z3T[dd][:, qs * 128:(qs + 1) * 128], tp[:])

        # t = z3 @ Wg1 + bg1 ; LN ; gelu -> d
        ts, stats = [], []
        for qs in range(QS):
            tps = d_ps.tile([128, D], F32, tag="tps", name="tps", bufs=2)
            for kk in range(2):
                nc.tensor.matmul(tps[:], lhsT=z3T[kk][:, qs * 128:(qs + 1) * 128],
                                 rhs=wg1[:, kk * D:(kk + 1) * D],
                                 start=(kk == 0), stop=False)
            nc.tensor.matmul(tps[:], lhsT=ones1[:], rhs=bg1_row[:],
                             start=False, stop=True)
            x = dec_sb.tile([128, D], F32, tag=f"dx{qs}", name=f"dx{qs}")
            sum_x = dec_sb.tile([128, 1], F32, tag=f"dsx{qs}", name=f"dsx{qs}")
            nc.scalar.activation(x[:], tps[:], AF.Copy, accum_out=sum_x[:])
            scratch = dec_sb.tile([128, D], F32, tag="dscr", name="dscr")
            sum_x2 = dec_sb.tile([128, 1], F32, tag=f"dsx2{qs}", name=f"dsx2{qs}")
            nc.scalar.activation(scratch[:], x[:], AF.Square, accum_out=sum_x2[:])
            ts.append(x)
            stats.append((sum_x, sum_x2))
        rstds = []
        for qs in range(QS):
            sum_x, sum_x2 = stats[qs]
            m = dec_sb.tile([128, 1], F32, tag=f"dm{qs}", name=f"dm{qs}")
            nc.vector.tensor_scalar(m[:], sum_x[:], 1.0 / D, None, ALU.mult)
            msq = dec_sb.tile([128, 1], F32, tag="dmsq", name="dmsq")
            nc.vector.tensor_tensor(msq[:], m[:], m[:], ALU.mult)
            var = dec_sb.tile([128, 1], F32, tag=f"dvar{qs}", name=f"dvar{qs}")
            nc.vector.tensor_scalar(var[:], sum_x2[:], 1.0 / D, msq[:],
                                    ALU.mult, ALU.subtract)
            lv = dec_sb.tile([128, 1], F32, tag=f"dlv{qs}", name=f"dlv{qs}")
            nc.scalar.activation(lv[:], var[:], AF.Ln, bias=eps5_c[:])
            rstd = dec_sb.tile([128, 1], F32, tag=f"drstd{qs}", name=f"drstd{qs}")
            nc.scalar.activation(rstd[:], lv[:], AF.Exp, scale=-0.5)
            rstds.append((m, rstd))
        for qs in range(QS):
            m, rstd = rstds[qs]
            xc = dec_sb.tile([128, D], F32, tag="dxc", name="dxc")
            nc.vector.tensor_scalar(xc[:], ts[qs][:], m[:], rstd[:],
                                    ALU.subtract, ALU.mult)
            t2 = dec_sb.tile([128, D], F32, tag="dt2", name="dt2")
            nc.vector.tensor_tensor(t2[:], xc[:], ln_b["lng_g"][:], ALU.mult)
            t3 = dec_sb.tile([128, D], F32, tag="dt3", name="dt3")
            nc.vector.tensor_tensor(t3[:], t2[:], ln_b["lng_b"][:], ALU.add)
            dtok = dec_sb.tile([128, D], F32, tag="dtok", name="dtok")
            nc.scalar.activation(dtok[:], t3[:], AF.Gelu)
            for dd in range(2):
                tp = tp3_ps.tile([128, 128], F32, tag="tp3", name="tp3")
                nc.tensor.transpose(tp[:], dtok[:, dd * 128:(dd + 1) * 128], ident[:])
                nc.vector.tensor_copy(dT[dd][:, qs * 128:(qs + 1) * 128], tp[:])

        # functional head: g = sigmoid(gelu(z3 @ Wf1 + bf1) @ Wf2 + bf2)
        wf1 = dec_sb.tile([128, 2 * 64], F32R, tag="wf1", name="wf1")
        for kk in range(2):
            dmar(wf1[:, kk * 64:(kk + 1) * 64], io["Wf1"][kk * 128:(kk + 1) * 128, :])
        wf2 = dec_sb.tile([64, 1], F32, tag="wf2", name="wf2")
        dma(wf2[:], io["Wf2"][:])
        fps = d_ps.tile([64, S], F32, tag="fps", name="fps", bufs=1)
        for kk in range(2):
            nc.tensor.matmul(fps[:], lhsT=wf1[:, kk * 64:(kk + 1) * 64],
                             rhs=z3T[kk][:], start=(kk == 0), stop=(kk == 1))
        fg = dec_sb.tile([64, S], F32, tag="fg", name="fg")
        nc.scalar.activation(fg[:], fps[:], AF.Gelu, bias=bf1_c[:])
        for qs in range(QS):
            gps = d_ps.tile([128, 1], F32, tag="gps", name="gps", bufs=1)
            nc.tensor.matmul(gps[:], lhsT=fg[:, qs * 128:(qs + 1) * 128], rhs=wf2[:],
                             start=True, stop=False)
            nc.tensor.matmul(gps[:], lhsT=ones1[:], rhs=bf2_row[:],
                             start=False, stop=True)
            ge = dec_sb.tile([128, 1], F32, tag="ge", name="ge")
            nc.scalar.activation(ge[:], gps[:], AF.Exp, scale=-1.0)
            gd = dec_sb.tile([128, 1], F32, tag="gd", name="gd")
            nc.vector.tensor_scalar(gd[:], ge[:], 1.0, None, ALU.add)
            gsb = dec_sb.tile([128, 1], F32, tag="gsb", name="gsb")
            nc.vector.reciprocal(gsb[:], gd[:])
            dma(io["g_out"][qs * 128:(qs + 1) * 128, :], gsb[:])

        tp3_cm.__exit__(None, None, None)
        dps_cm.__exit__(None, None, None)

        # preds = d @ Wg2 + bg2 ; softplus = ln(1+exp) ; +1e-6 on theta cols
        with pool("pp_ps", 3, space="PSUM") as pp_ps:
            for ft in range(8):
                f0 = ft * 512
                fw = min(512, 2 * G - f0)
                wg2t = dec_w.tile([128, 2 * 512], F32R, tag="wg2t", name="wg2t")
                for kk in range(2):
                    dmar(wg2t[:, kk * 512:kk * 512 + fw],
                         io["Wg2"][kk * 128:(kk + 1) * 128, f0:f0 + fw])
                for qs in range(QS):
                    pps = pp_ps.tile([128, 512], F32, tag="pps", name="pps")
                    for kk in range(2):
                        nc.tensor.matmul(pps[:, 0:fw],
                                         lhsT=dT[kk][:, qs * 128:(qs + 1) * 128],
                                         rhs=wg2t[:, kk * 512:kk * 512 + fw],
                                         start=(kk == 0), stop=False)
                    nc.tensor.matmul(pps[:, 0:fw], lhsT=ones1[:],
                                     rhs=bg2_row[0:1, f0:f0 + fw],
                                     start=False, stop=True)
                    esb = out_sb.tile([128, 512], F32, tag="esb", name="esb")
                    nc.scalar.activation(esb[:, 0:fw], pps[:, 0:fw], AF.Exp)
                    osb = out_sb.tile([128, 512], F32, tag="osb", name="osb")
                    nc.scalar.activation(osb[:, 0:fw], esb[:, 0:fw], AF.Ln, bias=1.0)
                    nc.vector.tensor_tensor(osb[:, 0:fw], osb[:, 0:fw], eps_t[:, 0:fw],
                                            ALU.add)
                    dma(io["preds_out"][qs * 128:(qs + 1) * 128, f0:f0 + fw],
                        osb[:, 0:fw])

    es.close()


def build_program():
    nc = bacc.Bacc("TRN2", target_bir_lowering=False, debug=False,
                   num_devices=NCORES)
    io = {}

    def inp(name, shape):
        io[name] = nc.dram_tensor(name, list(shape), F32, kind="ExternalInput").ap()

    inp("visT", [DU, N])
    inp("posT", [3, N])
    inp("gradT", [1, S])
    inp("B_fourier", [3, MAP])
    inp("W_img", [DU, D]); inp("b_img", [D])
    inp("W_pos", [2 * MAP, D]); inp("b_pos", [D])
    for w in ("Wq", "Wk", "Wv", "Wo"):
        inp(w, [D, D])
    for b in ("bq", "bk", "bv", "bo"):
        inp(b, [D])
    inp("ln1_g", [D]); inp("ln1_b", [D])
    inp("W_router", [D + 1, E]); inp("b_router", [E])
    inp("W1e", [E, D, 4 * D]); inp("b1e", [E, 4 * D])
    inp("W2e", [E, 4 * D, D]); inp("b2e", [E, D])
    inp("Wg1", [D, D]); inp("bg1", [D])
    inp("lng_g", [D]); inp("lng_b", [D])
    inp("Wg2", [D, 2 * G]); inp("bg2", [2 * G])
    inp("Wf1", [D, 64]); inp("bf1", [64])
    inp("Wf2", [64, 1]); inp("bf2", [1])
    io["preds_out"] = nc.dram_tensor("preds_out", [S, 2 * G], F32,
                                     kind="ExternalOutput").ap()
    io["g_out"] = nc.dram_tensor("g_out", [S, 1], F32, kind="ExternalOutput").ap()

    with tile.TileContext(nc) as tc:
        _build_body(nc, tc, io)
    nc.compile()
    return nc


_PROGRAM = None


def get_program():
    global _PROGRAM
    if _PROGRAM is None:
        _PROGRAM = build_program()
    return _PROGRAM


def prep_inputs(inputs):
    """Build the 8 per-core input maps (token-rotated full inputs)."""
    f = lambda k: np.ascontiguousarray(np.asarray(inputs[k], dtype=np.float32))
    vis_T = f("vis").T.copy()          # [1024, 4096]
    pos_T = f("pos").T.copy()          # [3, 4096]
    grad = f("grad")                   # [4096, 1]
    shared = {}
    for k in ("B_fourier", "W_img", "b_img", "W_pos", "b_pos", "Wq", "bq", "Wk",
              "bk", "Wv", "bv", "Wo", "bo", "ln1_g", "ln1_b", "W_router",
              "b_router", "W1e", "b1e", "W2e", "b2e", "Wg1", "bg1", "lng_g",
              "lng_b", "Wg2", "bg2", "Wf1", "bf1", "Wf2", "bf2"):
        shared[k] = f(k)
    in_maps = []
    for c in range(NCORES):
        o = c * S
        m = dict(shared)
        m["visT"] = np.ascontiguousarray(np.roll(vis_T, -o, axis=1))
        m["posT"] = np.ascontiguousarray(np.roll(pos_T, -o, axis=1))
        m["gradT"] = np.ascontiguousarray(grad[o:o + S, 0][None, :])
        in_maps.append(m)
    return in_maps


def kernel(**inputs):
    nc = get_program()
    in_maps = prep_inputs(inputs)
    res = run_bass_kernel_spmd(nc, in_maps, core_ids=list(range(NCORES)))
    preds = np.concatenate([res.results[c]["preds_out"] for c in range(NCORES)], 0)
    g = np.concatenate([res.results[c]["g_out"] for c in range(NCORES)], 0)
    preds = preds.reshape(N, G, 2)
    mu = np.ascontiguousarray(preds[:, :, 0])
    theta = np.ascontiguousarray(preds[:, :, 1])
    return mu, theta, g


# revision 30
# speedup vs baseline: 2.2207x; 1.2056x over previous
"""Trainium2 Bass kernel for nn_MoEST_Plus (MoE spatial transformer).

Sharding: data-parallel over the N (spot) axis. Each of the 8 cores
receives a token-rotated copy of the full inputs so that its OWN 512
tokens sit at positions 0..511; the encoder (z) is computed for all 4096
tokens on every core (attention needs full K/V; replication avoids
collectives), while attention-queries / MoE / decoders run only on the
core's own 512 tokens. Outputs are gathered and de-interleaved on host.

Precision: heavy matmuls run in float32r (1 cycle/row vs 4 for fp32).
The router-feeding path (z for the core's own tokens -> LN1 -> router
logits) is kept in full fp32 so top-1 expert selection matches the
reference even for near-tied router probabilities.
"""

import os
import sys

import numpy as np

for _p in ("/opt/trn_rl_repo",):
    if os.path.isdir(_p) and _p not in sys.path:
        sys.path.insert(0, _p)

import concourse.bacc as bacc
import concourse.mybir as mybir
import concourse.tile as tile
from concourse import masks
from concourse.bass_utils import run_bass_kernel_spmd

AF = mybir.ActivationFunctionType
ALU = mybir.AluOpType
F32 = mybir.dt.float32
F32R = mybir.dt.float32r

N = 4096          # tokens (spots)
DU = 1024         # dim_uni
D = 256           # dim_hidden
G = 2000          # genes
E = 4             # experts
H = 4             # heads
HD = 64           # head dim
MAP = 64          # fourier mapping size
NCORES = 8
S = N // NCORES   # own tokens per core = 512
QS = S // 128     # 128-token q tiles = 4
TT = N // 512     # 512-token tiles = 8
KT = N // 128     # 128-token key tiles = 32
PI = float(np.pi)
MAGIC = 12582912.0  # 1.5 * 2**23 fp32 round-to-nearest constant


def _build_body(nc, tc, io):
    from contextlib import ExitStack

    es = ExitStack()

    def pool(name, bufs, space="SBUF"):
        return tc.tile_pool(name=name, bufs=bufs, space=space)

    def dma(dst, src):
        nc.sync.dma_start(dst, src)

    def dmar(dst, src):
        """DMA fp32 DRAM -> f32r SBUF tile (hardware rounds)."""
        nc.sync.dma_start(dst, src.bitcast(F32R))

    # ---------------- constants / small params ----------------
    cpool = es.enter_context(tc.tile_pool(name="consts", bufs=1))
    ident = cpool.tile([128, 128], F32, tag="ident", name="ident")
    masks.make_identity(nc, ident[:])
    ones1 = cpool.tile([1, 128], F32, tag="ones1", name="ones1")
    nc.gpsimd.memset(ones1[:], 1.0)
    ones_b = cpool.tile([128, D], F32, tag="ones_b", name="ones_b")
    nc.gpsimd.memset(ones_b[:], 1.0)
    eps5_c = cpool.tile([128, 1], F32, tag="eps5_c", name="eps5_c")
    nc.gpsimd.memset(eps5_c[:], 1e-5)

    def row(dram_1d, n, tag):
        t = cpool.tile([1, n], F32, tag=tag, name=tag)
        dma(t[:], dram_1d.rearrange("(o x) -> o x", o=1))
        return t

    def col(dram_1d, lo, n, tag):
        t = cpool.tile([n, 1], F32, tag=tag, name=tag)
        dma(t[:], dram_1d[lo:lo + n].rearrange("(p o) -> p o", o=1))
        return t

    bimg_c = [col(io["b_img"], dd * 128, 128, f"bimg{dd}") for dd in range(2)]
    bpos_c = [col(io["b_pos"], dd * 128, 128, f"bpos{dd}") for dd in range(2)]
    bz_c = []
    for dd in range(2):
        t = cpool.tile([128, 1], F32, tag=f"bz{dd}", name=f"bz{dd}")
        nc.vector.tensor_add(t[:], bimg_c[dd][:], bpos_c[dd][:])
        bz_c.append(t)
    bk_c = [col(io["bk"], dd * 128, 128, f"bk{dd}") for dd in range(2)]
    bq_c = [col(io["bq"], dd * 128, 128, f"bq{dd}") for dd in range(2)]
    bf1_c = col(io["bf1"], 0, 64, "bf1")

    bv_row = row(io["bv"], D, "bvr")
    bo_row = row(io["bo"], D, "bor")
    brt_row = row(io["b_router"], E, "brtr")
    bg1_row = row(io["bg1"], D, "bg1r")
    bf2_row = row(io["bf2"], 1, "bf2r")

    # LN gamma/beta broadcast tiles [128, 256] via PE outer-product
    ln_rows = {k: row(io[k], D, k + "r") for k in ("ln1_g", "ln1_b", "lng_g", "lng_b")}
    ln_b = {}
    with pool("lnb_ps", 2, space="PSUM") as lnps:
        for k, r in ln_rows.items():
            ps = lnps.tile([128, D], F32, tag="lnb", name="lnb")
            nc.tensor.matmul(ps[:], lhsT=ones1[:], rhs=r[:], start=True, stop=True)
            t = cpool.tile([128, D], F32, tag=k + "b", name=k + "b")
            nc.vector.tensor_copy(t[:], ps[:])
            ln_b[k] = t

    # eps tile for theta += 1e-6 (odd output columns)
    eps_t = cpool.tile([128, 1024], F32, tag="epst", name="epst")
    nc.gpsimd.memset(eps_t[:], 0.0)
    nc.gpsimd.memset(eps_t[:].rearrange("p (g two) -> p g two", two=2)[:, :, 1:2], 1e-6)

    bfour = cpool.tile([3, MAP], F32, tag="bfour", name="bfour")
    dma(bfour[:], io["B_fourier"][:])

    # ---------------- persistent activation tiles ----------------
    act_pool = es.enter_context(tc.tile_pool(name="acts", bufs=1))
    z_own = act_pool.tile([128, QS * D], F32, tag="z_own", name="z_own")
    z2_sb = act_pool.tile([128, QS * D], F32, tag="z2", name="z2")
    z2T = [act_pool.tile([128, S], F32, tag=f"z2T{dd}", name=f"z2T{dd}") for dd in range(2)]
    z2Tr = [act_pool.tile([128, S], F32R, tag=f"z2Tr{dd}", name=f"z2Tr{dd}") for dd in range(2)]
    z3_sb = act_pool.tile([128, QS * D], F32, tag="z3", name="z3")
    gate_sb = act_pool.tile([128, QS * E], F32, tag="gate", name="gate")

    # =====================================================================
    # Phase 1+2: encoder (z_T full f32r + fp32 own slice) then k/v/q
    # =====================================================================
    zT_cm = tc.tile_pool(name="zT_pool", bufs=1)
    zT_pool = zT_cm.__enter__()
    zT = [zT_pool.tile([128, N], F32R, tag=f"zT{dd}", name=f"zT{dd}") for dd in range(2)]
    z_ownT = [zT_pool.tile([128, S], F32, tag=f"z_ownT{dd}", name=f"z_ownT{dd}")
              for dd in range(2)]

    encw_cm = tc.tile_pool(name="enc_sb", bufs=1)
    enc_sb = encw_cm.__enter__()
    with pool("vis_sb", 6) as vis_pool, \
         pool("four_sb", 2) as four_pool, \
         pool("enc_ps", 4, space="PSUM") as enc_ps, \
         pool("fours_ps", 2, space="PSUM") as four_ps:
        wimg_r = enc_sb.tile([128, 8 * D], F32R, tag="wimg_r", name="wimg_r")
        wimg = enc_sb.tile([128, 8 * D], F32, tag="wimg", name="wimg")
        for kk in range(8):
            dmar(wimg_r[:, kk * D:(kk + 1) * D], io["W_img"][kk * 128:(kk + 1) * 128, :])
            dma(wimg[:, kk * D:(kk + 1) * D], io["W_img"][kk * 128:(kk + 1) * 128, :])
        wpos_r = enc_sb.tile([128, D], F32R, tag="wpos_r", name="wpos_r")
        dmar(wpos_r[:], io["W_pos"][:])
        wpos = enc_sb.tile([128, D], F32, tag="wpos", name="wpos")
        dma(wpos[:], io["W_pos"][:])

        for tt in range(TT):
            t0 = tt * 512
            # fourier features (fp32, range-reduced)
            pos_sb = four_pool.tile([3, 512], F32, tag="pos", name="pos")
            dma(pos_sb[:], io["posT"][:, t0:t0 + 512])
            xp = four_ps.tile([MAP, 512], F32, tag="xp", name="xp")
            nc.tensor.matmul(xp[:], lhsT=bfour[:], rhs=pos_sb[:], start=True, stop=True)
            r1 = four_pool.tile([MAP, 512], F32, tag="r1", name="r1")
            nc.vector.tensor_scalar(r1[:], xp[:], MAGIC, -MAGIC, ALU.add, ALU.add)
            fx = four_pool.tile([128, 512], F32, tag="fx", name="fx")
            nc.vector.tensor_tensor(fx[0:MAP, :], xp[:], r1[:], ALU.subtract)
            t2 = four_pool.tile([MAP, 512], F32, tag="t2", name="t2")
            nc.vector.tensor_scalar(t2[:], xp[:], 0.25, None, ALU.add)
            r2 = four_pool.tile([MAP, 512], F32, tag="r2", name="r2")
            nc.vector.tensor_scalar(r2[:], t2[:], MAGIC, -MAGIC, ALU.add, ALU.add)
            nc.vector.tensor_tensor(fx[MAP:128, :], t2[:], r2[:], ALU.subtract)
            four = four_pool.tile([128, 512], F32, tag="four", name="four")
            nc.scalar.activation(four[:], fx[:], AF.Sin, scale=2 * PI)
            four_r = four_pool.tile([128, 512], F32R, tag="four_rr", name="four_rr")
            nc.vector.tensor_copy(four_r[:], four[:])

            vtiles = []
            for kk in range(8):
                vt = vis_pool.tile([128, 512], F32R, tag="vis", name="vis")
                dmar(vt[:], io["visT"][kk * 128:(kk + 1) * 128, t0:t0 + 512])
                vtiles.append(vt)

            for dd in range(2):
                zps = enc_ps.tile([128, 512], F32, tag="zps", name="zps")
                for kk in range(8):
                    nc.tensor.matmul(
                        zps[:], lhsT=wimg_r[:, kk * D + dd * 128:kk * D + dd * 128 + 128],
                        rhs=vtiles[kk][:], start=(kk == 0), stop=False)
                nc.tensor.matmul(
                    zps[:], lhsT=wpos_r[:, dd * 128:dd * 128 + 128],
                    rhs=four_r[:], start=False, stop=True)
                nc.vector.tensor_scalar(zT[dd][:, t0:t0 + 512], zps[:], bz_c[dd][:], None, ALU.add)

            if tt == 0:
                # fp32 own-slice z (feeds LN1 -> router; keeps routing faithful)
                votiles = []
                for kk in range(8):
                    vo = vis_pool.tile([128, 512], F32, tag="vis_o", name="vis_o")
                    dma(vo[:], io["visT"][kk * 128:(kk + 1) * 128, 0:512])
                    votiles.append(vo)
                for dd in range(2):
                    zops = enc_ps.tile([128, 512], F32, tag="zops", name="zops", bufs=2)
                    for kk in range(8):
                        nc.tensor.matmul(
                            zops[:], lhsT=wimg[:, kk * D + dd * 128:kk * D + dd * 128 + 128],
                            rhs=votiles[kk][:], start=(kk == 0), stop=False)
                    nc.tensor.matmul(
                        zops[:], lhsT=wpos[:, dd * 128:dd * 128 + 128],
                        rhs=four[:], start=False, stop=True)
                    nc.vector.tensor_scalar(z_ownT[dd][:], zops[:], bz_c[dd][:], None, ALU.add)

    encw_cm.__exit__(None, None, None)

    attnio_cm = tc.tile_pool(name="attn_io", bufs=1)
    attn_io = attnio_cm.__enter__()
    kT = [attn_io.tile([128, N], F32R, tag=f"kT{dd}", name=f"kT{dd}") for dd in range(2)]
    v_sb = attn_io.tile([128, KT * (H * (HD + 1))], F32R, tag="v1", name="v1")
    qT = [attn_io.tile([128, S], F32R, tag=f"qT{dd}", name=f"qT{dd}") for dd in range(2)]
    _vones = v_sb[:].rearrange("p (t c) -> p t c", c=HD + 1)[:, :, HD]
    nc.vector.tensor_copy(_vones, ones_b[:, 0:KT * H])

    if True:
        with pool("kvq_sb", 1) as kvq_sb, \
             pool("kvq_ps", 3, space="PSUM") as kvq_ps, \
             pool("tp_ps", 2, space="PSUM") as tp_ps:
            wk = kvq_sb.tile([128, 2 * D], F32R, tag="wk", name="wk")
            wq = kvq_sb.tile([128, 2 * D], F32R, tag="wq", name="wq")
            wv = kvq_sb.tile([128, 2 * D], F32R, tag="wv", name="wv")
            for kk in range(2):
                dmar(wk[:, kk * D:(kk + 1) * D], io["Wk"][kk * 128:(kk + 1) * 128, :])
                dmar(wq[:, kk * D:(kk + 1) * D], io["Wq"][kk * 128:(kk + 1) * 128, :])
                dmar(wv[:, kk * D:(kk + 1) * D], io["Wv"][kk * 128:(kk + 1) * 128, :])

            # k_T (feature-major, all tokens)
            for dd in range(2):
                for tt in range(TT):
                    t0 = tt * 512
                    kps = kvq_ps.tile([128, 512], F32, tag="kps", name="kps")
                    for kk in range(2):
                        nc.tensor.matmul(
                            kps[:], lhsT=wk[:, kk * D + dd * 128:kk * D + dd * 128 + 128],
                            rhs=zT[kk][:, t0:t0 + 512], start=(kk == 0), stop=(kk == 1))
                    nc.vector.tensor_scalar(kT[dd][:, t0:t0 + 512], kps[:], bk_c[dd][:], None, ALU.add)

            # q_T (feature-major, own tokens)
            for dd in range(2):
                qps = kvq_ps.tile([128, 512], F32, tag="kps", name="qps")
                for kk in range(2):
                    nc.tensor.matmul(
                        qps[:], lhsT=wq[:, kk * D + dd * 128:kk * D + dd * 128 + 128],
                        rhs=zT[kk][:, 0:512], start=(kk == 0), stop=(kk == 1))
                nc.vector.tensor_scalar(qT[dd][:], qps[:], bq_c[dd][:], None, ALU.add)

            # v (token-major, all tokens) with bias via ones-row
            for kt in range(KT):
                p0 = kt * 128
                vps = kvq_ps.tile([128, D], F32, tag="vps", name="vps")
                for kk in range(2):
                    nc.tensor.matmul(
                        vps[:], lhsT=zT[kk][:, p0:p0 + 128],
                        rhs=wv[:, kk * D:(kk + 1) * D], start=(kk == 0), stop=False)
                nc.tensor.matmul(vps[:], lhsT=ones1[:], rhs=bv_row[:],
                                 start=False, stop=True)
                dst = v_sb[:, kt * H * (HD + 1):(kt + 1) * H * (HD + 1)]
                dst = dst.rearrange("p (h c) -> p h c", c=HD + 1)[:, :, 0:HD]
                nc.vector.tensor_copy(dst, vps[:].rearrange("p (h c) -> p h c", c=HD))

            # z_own (token-major, fp32) from fp32 z_ownT
            for qs in range(QS):
                for dd in range(2):
                    tp = tp_ps.tile([128, 128], F32, tag="tp", name="tp")
                    nc.tensor.transpose(tp[:], z_ownT[dd][:, qs * 128:(qs + 1) * 128],
                                        ident[:])
                    nc.vector.tensor_copy(
                        z_own[:, qs * D + dd * 128:qs * D + dd * 128 + 128], tp[:])

    # =====================================================================
    # Phase 3: attention (own 512 queries over all 4096 keys)
    # =====================================================================
    with pool("attn_sb", 1) as attn_sb, \
         pool("p_sb", 4) as p_pool:
        wo_sb = attn_sb.tile([HD, H * D], F32R, tag="wo", name="wo")
        for h in range(H):
            dmar(wo_sb[:, h * D:(h + 1) * D], io["Wo"][h * HD:(h + 1) * HD, :])
        grad_sb = attn_sb.tile([1, S], F32, tag="grad", name="grad")
        dma(grad_sb[:], io["gradT"][:])
        wr_a = attn_sb.tile([128, E], F32, tag="wra", name="wra")
        wr_b = attn_sb.tile([128, E], F32, tag="wrb", name="wrb")
        wr_c = attn_sb.tile([1, E], F32, tag="wrc", name="wrc")
        dma(wr_a[:], io["W_router"][0:128, :])
        dma(wr_b[:], io["W_router"][128:256, :])
        dma(wr_c[:], io["W_router"][256:257, :])
        ctxn = [attn_sb.tile([HD, S], F32R, tag=f"ctxn{h}", name=f"ctxn{h}")
                for h in range(H)]
        heads_ps_cm = [pool("s_ps", 2, space="PSUM"), pool("ctx_ps", 2, space="PSUM")]
        s_ps = heads_ps_cm[0].__enter__()
        ctx_ps_pool = heads_ps_cm[1].__enter__()
        for h in range(H):
            dd, off = h // 2, (h % 2) * HD
            ctx_ps = ctx_ps_pool.tile([HD + 1, S], F32, tag="ctx", name="ctx")
            for kp in range(KT // 2):
                sps = s_ps.tile([128, 2 * S], F32, tag="sps", name="sps")
                for half in range(2):
                    kt = 2 * kp + half
                    nc.tensor.matmul(
                        sps[:, half * S:(half + 1) * S],
                        lhsT=kT[dd][off:off + HD, kt * 128:(kt + 1) * 128],
                        rhs=qT[dd][off:off + HD, :], start=True, stop=True)
                pt = p_pool.tile([128, 2 * S], F32R, tag="pt", name="pt")
                nc.scalar.activation(pt[:], sps[:], AF.Exp, scale=1.0 / 8.0)
                for half in range(2):
                    kt = 2 * kp + half
                    c0 = kt * H * (HD + 1) + h * (HD + 1)
                    nc.tensor.matmul(ctx_ps[:], lhsT=v_sb[:, c0:c0 + HD + 1],
                                     rhs=pt[:, half * S:(half + 1) * S],
                                     start=(kt == 0), stop=(kt == KT - 1))
            rd = attn_sb.tile([1, S], F32, tag="rd", name="rd")
            nc.vector.reciprocal(rd[:], ctx_ps[HD:HD + 1, :])
            rb_ps = s_ps.tile([HD, S], F32, tag="rbps", name="rbps", bufs=1)
            nc.tensor.matmul(rb_ps[:], lhsT=ones1[0:1, 0:HD], rhs=rd[:],
                             start=True, stop=True)
            rb = attn_sb.tile([HD, S], F32, tag="rb", name="rb")
            nc.vector.tensor_copy(rb[:], rb_ps[:])
            nc.vector.tensor_tensor(ctxn[h][:], ctx_ps[0:HD, :], rb[:], ALU.mult)

        heads_ps_cm[1].__exit__(None, None, None)
        heads_ps_cm[0].__exit__(None, None, None)

        # attn_out + residual + LN1 + router + gates
        with pool("wo_ps", 2, space="PSUM") as wo_ps, \
             pool("ln_sb", 1) as ln_sb, \
             pool("r_ps", 2, space="PSUM") as r_ps, \
             pool("tp2_ps", 2, space="PSUM") as tp2_ps:
            xs, stats = [], []
            for qs in range(QS):
                aps = wo_ps.tile([128, D], F32, tag="aps", name="aps")
                for h in range(H):
                    nc.tensor.matmul(
                        aps[:], lhsT=ctxn[h][:, qs * 128:(qs + 1) * 128],
                        rhs=wo_sb[:, h * D:(h + 1) * D],
                        start=(h == 0), stop=False)
                nc.tensor.matmul(aps[:], lhsT=ones1[:], rhs=bo_row[:],
                                 start=False, stop=True)
                x = ln_sb.tile([128, D], F32, tag=f"x{qs}", name=f"x{qs}")
                nc.vector.tensor_tensor(x[:], z_own[:, qs * D:(qs + 1) * D], aps[:],
                                        ALU.add)
                sum_x = ln_sb.tile([128, 1], F32, tag=f"sx{qs}", name=f"sx{qs}")
                scratch = ln_sb.tile([128, D], F32, tag="lnscr", name="lnscr")
                nc.scalar.activation(scratch[:], x[:], AF.Copy, accum_out=sum_x[:])
                sum_x2 = ln_sb.tile([128, 1], F32, tag=f"sx2{qs}", name=f"sx2{qs}")
                nc.scalar.activation(scratch[:], x[:], AF.Square, accum_out=sum_x2[:])
                xs.append(x)
                stats.append((sum_x, sum_x2))
            rstds = []
            for qs in range(QS):
                sum_x, sum_x2 = stats[qs]
                m = ln_sb.tile([128, 1], F32, tag=f"m{qs}", name=f"m{qs}")
                nc.vector.tensor_scalar(m[:], sum_x[:], 1.0 / D, None, ALU.mult)
                msq = ln_sb.tile([128, 1], F32, tag=f"msq{qs}", name=f"msq{qs}")
                nc.vector.tensor_tensor(msq[:], m[:], m[:], ALU.mult)
                var = ln_sb.tile([128, 1], F32, tag=f"var{qs}", name=f"var{qs}")
                nc.vector.tensor_scalar(var[:], sum_x2[:], 1.0 / D, msq[:],
                                        ALU.mult, ALU.subtract)
                lv = ln_sb.tile([128, 1], F32, tag=f"lv{qs}", name=f"lv{qs}")
                nc.scalar.activation(lv[:], var[:], AF.Ln, bias=eps5_c[:])
                rstd = ln_sb.tile([128, 1], F32, tag=f"rstd{qs}", name=f"rstd{qs}")
                nc.scalar.activation(rstd[:], lv[:], AF.Exp, scale=-0.5)
                rstds.append((m, rstd))
            for qs in range(QS):
                m, rstd = rstds[qs]
                x = xs[qs]
                xc = ln_sb.tile([128, D], F32, tag="xc", name="xc")
                nc.vector.tensor_scalar(xc[:], x[:], m[:], rstd[:],
                                        ALU.subtract, ALU.mult)
                t2 = ln_sb.tile([128, D], F32, tag="t2l", name="t2l")
                nc.vector.tensor_tensor(t2[:], xc[:], ln_b["ln1_g"][:], ALU.mult)
                nc.vector.tensor_tensor(
                    z2_sb[:, qs * D:(qs + 1) * D], t2[:], ln_b["ln1_b"][:], ALU.add)
                for dd in range(2):
                    tp = tp2_ps.tile([128, 128], F32, tag="tp2", name="tp2")
                    nc.tensor.transpose(
                        tp[:], z2_sb[:, qs * D + dd * 128:qs * D + dd * 128 + 128],
                        ident[:])
                    nc.vector.tensor_copy(z2T[dd][:, qs * 128:(qs + 1) * 128], tp[:])
                    nc.vector.tensor_copy(z2Tr[dd][:, qs * 128:(qs + 1) * 128], tp[:])

            # router logits + softmax + top-1 gate (fp32)
            lps_l, pr_l = [], []
            for qs in range(QS):
                lps = r_ps.tile([128, E], F32, tag="lps", name="lps")
                nc.tensor.matmul(lps[:], lhsT=z2T[0][:, qs * 128:(qs + 1) * 128],
                                 rhs=wr_a[:], start=True, stop=False)
                nc.tensor.matmul(lps[:], lhsT=z2T[1][:, qs * 128:(qs + 1) * 128],
                                 rhs=wr_b[:], start=False, stop=False)
                nc.tensor.matmul(lps[:], lhsT=grad_sb[0:1, qs * 128:(qs + 1) * 128],
                                 rhs=wr_c[:], start=False, stop=False)
                nc.tensor.matmul(lps[:], lhsT=ones1[:], rhs=brt_row[:],
                                 start=False, stop=True)
                nm = ln_sb.tile([128, 1], F32, tag=f"nm{qs}", name=f"nm{qs}")
                nc.vector.tensor_reduce(nm[:], lps[:], mybir.AxisListType.X, ALU.max,
                                        negate=True)
                lps_l.append(lps)
                pr_l.append(nm)
            for qs in range(QS):
                lps, nm = lps_l[qs], pr_l[qs]
                ex = ln_sb.tile([128, E], F32, tag=f"ex{qs}", name=f"ex{qs}")
                nc.scalar.activation(ex[:], lps[:], AF.Exp, bias=nm[:])
                se = ln_sb.tile([128, 1], F32, tag="se", name="se")
                nc.vector.tensor_reduce(se[:], ex[:], mybir.AxisListType.X, ALU.add)
                rse = ln_sb.tile([128, 1], F32, tag="rse", name="rse")
                nc.vector.reciprocal(rse[:], se[:])
                pr = ln_sb.tile([128, E], F32, tag="pr", name="pr")
                nc.vector.tensor_scalar(pr[:], ex[:], rse[:], None, ALU.mult)
                pm = ln_sb.tile([128, 1], F32, tag="pm", name="pm")
                nc.vector.tensor_reduce(pm[:], pr[:], mybir.AxisListType.X, ALU.max)
                mk = ln_sb.tile([128, E], F32, tag="mk", name="mk")
                nc.vector.tensor_scalar(mk[:], pr[:], pm[:], None, ALU.is_ge)
                nc.vector.tensor_tensor(
                    gate_sb[:, qs * E:(qs + 1) * E], pr[:], mk[:], ALU.mult)

    attnio_cm.__exit__(None, None, None)
    zT_cm.__exit__(None, None, None)

    # =====================================================================
    # Phase 4: MoE experts (dense f32r compute, gated combine)
    # =====================================================================
    with pool("moe_w", 2) as moe_w, \
         pool("moe_h", 2) as moe_h, \
         pool("moe_sb", 2) as moe_sb, \
         pool("h_ps", 2, space="PSUM") as h_ps, \
         pool("y_ps", 2, space="PSUM") as y_ps:
        b2e_sb = moe_sb.tile([1, E * D], F32, tag="b2e", name="b2e", bufs=1)
        for e in range(E):
            dma(b2e_sb[:, e * D:(e + 1) * D], io["b2e"][e:e + 1, :])
        acc = [moe_sb.tile([128, D], F32, tag=f"acc{qs}", name=f"acc{qs}")
               for qs in range(QS)]
        for e in range(E):
            w1t = moe_w.tile([128, 2 * 4 * D], F32R, tag="w1t", name="w1t")
            for kk in range(2):
                dmar(w1t[:, kk * 4 * D:(kk + 1) * 4 * D],
                     io["W1e"][e, kk * 128:(kk + 1) * 128, :])
            w2t = moe_w.tile([128, 8 * D], F32R, tag="w2t", name="w2t")
            for kk in range(8):
                dmar(w2t[:, kk * D:(kk + 1) * D],
                     io["W2e"][e, kk * 128:(kk + 1) * 128, :])
            b1c = moe_w.tile([128, 8], F32, tag="b1c", name="b1c")
            dma(b1c[:], io["b1e"][e].rearrange("(f p) -> p f", p=128))

            hsb = moe_h.tile([128, 8 * S], F32R, tag="hsb", name="hsb")
            for ft in range(8):
                hps = h_ps.tile([128, S], F32, tag="hps", name="hps")
                for kk in range(2):
                    nc.tensor.matmul(
                        hps[:],
                        lhsT=w1t[:, kk * 4 * D + ft * 128:kk * 4 * D + ft * 128 + 128],
                        rhs=z2Tr[kk][:], start=(kk == 0), stop=(kk == 1))
                nc.scalar.activation(hsb[:, ft * S:(ft + 1) * S], hps[:], AF.Gelu,
                                     bias=b1c[:, ft:ft + 1])
            for qs in range(QS):
                yps = y_ps.tile([128, D], F32, tag="yps", name="yps")
                for ft in range(8):
                    nc.tensor.matmul(
                        yps[:], lhsT=hsb[:, ft * S + qs * 128:ft * S + qs * 128 + 128],
                        rhs=w2t[:, ft * D:(ft + 1) * D], start=(ft == 0), stop=False)
                nc.tensor.matmul(yps[:], lhsT=ones1[:],
                                 rhs=b2e_sb[:, e * D:(e + 1) * D],
                                 start=False, stop=True)
                gt = moe_sb.tile([128, D], F32, tag="gt", name="gt")
                nc.vector.tensor_scalar(gt[:], yps[:],
                                        gate_sb[:, qs * E + e:qs * E + e + 1],
                                        None, ALU.mult)
                if e == 0:
                    nc.vector.tensor_copy(acc[qs][:], gt[:])
                else:
                    nc.vector.tensor_tensor(acc[qs][:], acc[qs][:], gt[:], ALU.add)
        for qs in range(QS):
            nc.vector.tensor_tensor(z3_sb[:, qs * D:(qs + 1) * D],
                                    z2_sb[:, qs * D:(qs + 1) * D], acc[qs][:], ALU.add)

    # =====================================================================
    # Phase 5: gene decoder + functional head
    # =====================================================================
    with pool("dec_sb", 1) as dec_sb, \
         pool("dec_w", 3) as dec_w, \
         pool("out_sb", 4) as out_sb:
        dps_cm = pool("d_ps", 1, space="PSUM")
        tp3_cm = pool("tp3_ps", 2, space="PSUM")
        d_ps = dps_cm.__enter__()
        tp3_ps = tp3_cm.__enter__()
        bg2_row = dec_sb.tile([1, 2 * G], F32, tag="bg2r", name="bg2r")
        dma(bg2_row[:], io["bg2"].rearrange("(o x) -> o x", o=1))
        z3T = [dec_sb.tile([128, S], F32R, tag=f"z3T{dd}", name=f"z3T{dd}")
               for dd in range(2)]
        dT = [dec_sb.tile([128, S], F32R, tag=f"dT{dd}", name=f"dT{dd}")
              for dd in range(2)]
        wg1 = dec_sb.tile([128, 2 * D], F32R, tag="wg1", name="wg1")
        for kk in range(2):
            dmar(wg1[:, kk * D:(kk + 1) * D], io["Wg1"][kk * 128:(kk + 1) * 128, :])
        for qs in range(QS):
            for dd in range(2):
                tp = tp3_ps.tile([128, 128], F32, tag="tp3", name="tp3")
                nc.tensor.transpose(
                    tp[:], z3_sb[:, qs * D + dd * 128:qs * D + dd * 128 + 128], ident[:])
                nc.vector.tensor_copy(z3T[dd][:, qs * 128:(qs + 1) * 128], tp[:])

        # t = z3 @ Wg1 + bg1 ; LN ; gelu -> d
        ts, stats = [], []
        for qs in range(QS):
            tps = d_ps.tile([128, D], F32, tag="tps", name="tps", bufs=2)
            for kk in range(2):
                nc.tensor.matmul(tps[:], lhsT=z3T[kk][:, qs * 128:(qs + 1) * 128],
                                 rhs=wg1[:, kk * D:(kk + 1) * D],
                                 start=(kk == 0), stop=False)
            nc.tensor.matmul(tps[:], lhsT=ones1[:], rhs=bg1_row[:],
                             start=False, stop=True)
            x = dec_sb.tile([128, D], F32, tag=f"dx{qs}", name=f"dx{qs}")
            sum_x = dec_sb.tile([128, 1], F32, tag=f"dsx{qs}", name=f"dsx{qs}")
            nc.scalar.activation(x[:], tps[:], AF.Copy, accum_out=sum_x[:])
            scratch = dec_sb.tile([128, D], F32, tag="dscr", name="dscr")
            sum_x2 = dec_sb.tile([128, 1], F32, tag=f"dsx2{qs}", name=f"dsx2{qs}")
            nc.scalar.activation(scratch[:], x[:], AF.Square, accum_out=sum_x2[:])
            ts.append(x)
            stats.append((sum_x, sum_x2))
        rstds = []
        for qs in range(QS):
            sum_x, sum_x2 = stats[qs]
            m = dec_sb.tile([128, 1], F32, tag=f"dm{qs}", name=f"dm{qs}")
            nc.vector.tensor_scalar(m[:], sum_x[:], 1.0 / D, None, ALU.mult)
            msq = dec_sb.tile([128, 1], F32, tag="dmsq", name="dmsq")
            nc.vector.tensor_tensor(msq[:], m[:], m[:], ALU.mult)
            var = dec_sb.tile([128, 1], F32, tag=f"dvar{qs}", name=f"dvar{qs}")
            nc.vector.tensor_scalar(var[:], sum_x2[:], 1.0 / D, msq[:],
                                    ALU.mult, ALU.subtract)
            lv = dec_sb.tile([128, 1], F32, tag=f"dlv{qs}", name=f"dlv{qs}")
            nc.scalar.activation(lv[:], var[:], AF.Ln, bias=eps5_c[:])
            rstd = dec_sb.tile([128, 1], F32, tag=f"drstd{qs}", name=f"drstd{qs}")
            nc.scalar.activation(rstd[:], lv[:], AF.Exp, scale=-0.5)
            rstds.append((m, rstd))
        for qs in range(QS):
            m, rstd = rstds[qs]
            xc = dec_sb.tile([128, D], F32, tag="dxc", name="dxc")
            nc.vector.tensor_scalar(xc[:], ts[qs][:], m[:], rstd[:],
                                    ALU.subtract, ALU.mult)
            t2 = dec_sb.tile([128, D], F32, tag="dt2", name="dt2")
            nc.vector.tensor_tensor(t2[:], xc[:], ln_b["lng_g"][:], ALU.mult)
            t3 = dec_sb.tile([128, D], F32, tag="dt3", name="dt3")
            nc.vector.tensor_tensor(t3[:], t2[:], ln_b["lng_b"][:], ALU.add)
            dtok = dec_sb.tile([128, D], F32, tag="dtok", name="dtok")
            nc.scalar.activation(dtok[:], t3[:], AF.Gelu)
            for dd in range(2):
                tp = tp3_ps.tile([128, 128], F32, tag="tp3", name="tp3")
                nc.tensor.transpose(tp[:], dtok[:, dd * 128:(dd + 1) * 128], ident[:])
                nc.vector.tensor_copy(dT[dd][:, qs * 128:(qs + 1) * 128], tp[:])

        # functional head: g = sigmoid(gelu(z3 @ Wf1 + bf1) @ Wf2 + bf2)
        wf1 = dec_sb.tile([128, 2 * 64], F32R, tag="wf1", name="wf1")
        for kk in range(2):
            dmar(wf1[:, kk * 64:(kk + 1) * 64], io["Wf1"][kk * 128:(kk + 1) * 128, :])
        wf2 = dec_sb.tile([64, 1], F32, tag="wf2", name="wf2")
        dma(wf2[:], io["Wf2"][:])
        fps = d_ps.tile([64, S], F32, tag="fps", name="fps", bufs=1)
        for kk in range(2):
            nc.tensor.matmul(fps[:], lhsT=wf1[:, kk * 64:(kk + 1) * 64],
                             rhs=z3T[kk][:], start=(kk == 0), stop=(kk == 1))
        fg = dec_sb.tile([64, S], F32, tag="fg", name="fg")
        nc.scalar.activation(fg[:], fps[:], AF.Gelu, bias=bf1_c[:])
        for qs in range(QS):
            gps = d_ps.tile([128, 1], F32, tag="gps", name="gps", bufs=1)
            nc.tensor.matmul(gps[:], lhsT=fg[:, qs * 128:(qs + 1) * 128], rhs=wf2[:],
                             start=True, stop=False)
            nc.tensor.matmul(gps[:], lhsT=ones1[:], rhs=bf2_row[:],
                             start=False, stop=True)
            ge = dec_sb.tile([128, 1], F32, tag="ge", name="ge")
            nc.scalar.activation(ge[:], gps[:], AF.Exp, scale=-1.0)
            gd = dec_sb.tile([128, 1], F32, tag="gd", name="gd")
            nc.vector.tensor_scalar(gd[:], ge[:], 1.0, None, ALU.add)
            gsb = dec_sb.tile([128, 1], F32, tag="gsb", name="gsb")
            nc.vector.reciprocal(gsb[:], gd[:])
            dma(io["g_out"][qs * 128:(qs + 1) * 128, :], gsb[:])

        tp3_cm.__exit__(None, None, None)
        dps_cm.__exit__(None, None, None)

        # preds = d @ Wg2 + bg2 ; softplus = ln(1+exp) ; +1e-6 on theta cols
        with pool("pp_ps", 3, space="PSUM") as pp_ps:
            for ft in range(8):
                f0 = ft * 512
                fw = min(512, 2 * G - f0)
                wg2t = dec_w.tile([128, 2 * 512], F32R, tag="wg2t", name="wg2t")
                for kk in range(2):
                    dmar(wg2t[:, kk * 512:kk * 512 + fw],
                         io["Wg2"][kk * 128:(kk + 1) * 128, f0:f0 + fw])
                for qs in range(QS):
                    pps = pp_ps.tile([128, 512], F32, tag="pps", name="pps")
                    for kk in range(2):
                        nc.tensor.matmul(pps[:, 0:fw],
                                         lhsT=dT[kk][:, qs * 128:(qs + 1) * 128],
                                         rhs=wg2t[:, kk * 512:kk * 512 + fw],
                                         start=(kk == 0), stop=False)
                    nc.tensor.matmul(pps[:, 0:fw], lhsT=ones1[:],
                                     rhs=bg2_row[0:1, f0:f0 + fw],
                                     start=False, stop=True)
                    esb = out_sb.tile([128, 512], F32, tag="esb", name="esb")
                    nc.scalar.activation(esb[:, 0:fw], pps[:, 0:fw], AF.Exp)
                    osb = out_sb.tile([128, 512], F32, tag="osb", name="osb")
                    nc.scalar.activation(osb[:, 0:fw], esb[:, 0:fw], AF.Ln, bias=1.0)
                    nc.vector.tensor_tensor(osb[:, 0:fw], osb[:, 0:fw], eps_t[:, 0:fw],
                                            ALU.add)
                    dma(io["preds_out"][qs * 128:(qs + 1) * 128, f0:f0 + fw],
                        osb[:, 0:fw])

    es.close()


def _optimize_act_table_loads(nc):
    """Rewrite InstLoadActFuncSet placement to a minimal greedy cover.

    The stock pass assigns each activation func its first containing table
    set, which makes alternating Exp/Ln (softplus) reload tables on every
    instruction (~1.3-2.7us each).  Sets like natural_log_exp_and_others
    cover both; choose, at each required load point, the set covering the
    longest upcoming run of activation funcs, and delete the now-redundant
    loads.
    """
    from concourse.bacc import get_activation_tables

    tabs = list(get_activation_tables(nc.m.arch).items())
    sets = [frozenset(funcs) for _name, funcs in tabs]
    for blk in nc.main_func.blocks:
        insts = blk.instructions
        positions = []   # (idx, kind, payload)
        for idx, inst in enumerate(insts):
            tn = type(inst).__name__
            if tn == "InstLoadActFuncSet":
                positions.append((idx, "load", inst))
            elif isinstance(inst, mybir.InstActivation):
                positions.append((idx, "act", inst.func))
        if not positions:
            continue
        # funcs after each position index in the stream
        funcs_stream = [p for p in positions if p[1] == "act"]
        to_delete = []
        current = None  # index into sets
        fi = 0  # pointer into funcs_stream
        for idx, kind, payload in positions:
            if kind == "act":
                fi += 1
                continue
            inst = payload
            upcoming = [f for _i, _k, f in funcs_stream[fi:]]
            if not upcoming:
                to_delete.append(inst)
                continue
            nxt = upcoming[0]
            if current is not None and nxt in sets[current]:
                si = inst.sync_info
                if si is not None and (si.on_wait or si.on_update):
                    inst.act_func_set_id = current  # keep (reload same set)
                else:
                    to_delete.append(inst)
                continue
            best, best_len = None, -1
            for s_id, s in enumerate(sets):
                if nxt not in s:
                    continue
                n = 0
                for f in upcoming:
                    if f in s:
                        n += 1
                    else:
                        break
                if n > best_len:
                    best, best_len = s_id, n
            inst.act_func_set_id = best
            current = best
        for inst in to_delete:
            insts.remove(inst)


def build_program():
    nc = bacc.Bacc("TRN2", target_bir_lowering=False, debug=False,
                   num_devices=NCORES)
    io = {}

    def inp(name, shape):
        io[name] = nc.dram_tensor(name, list(shape), F32, kind="ExternalInput").ap()

    inp("visT", [DU, N])
    inp("posT", [3, N])
    inp("gradT", [1, S])
    inp("B_fourier", [3, MAP])
    inp("W_img", [DU, D]); inp("b_img", [D])
    inp("W_pos", [2 * MAP, D]); inp("b_pos", [D])
    for w in ("Wq", "Wk", "Wv", "Wo"):
        inp(w, [D, D])
    for b in ("bq", "bk", "bv", "bo"):
        inp(b, [D])
    inp("ln1_g", [D]); inp("ln1_b", [D])
    inp("W_router", [D + 1, E]); inp("b_router", [E])
    inp("W1e", [E, D, 4 * D]); inp("b1e", [E, 4 * D])
    inp("W2e", [E, 4 * D, D]); inp("b2e", [E, D])
    inp("Wg1", [D, D]); inp("bg1", [D])
    inp("lng_g", [D]); inp("lng_b", [D])
    inp("Wg2", [D, 2 * G]); inp("bg2", [2 * G])
    inp("Wf1", [D, 64]); inp("bf1", [64])
    inp("Wf2", [64, 1]); inp("bf2", [1])
    io["preds_out"] = nc.dram_tensor("preds_out", [S, 2 * G], F32,
                                     kind="ExternalOutput").ap()
    io["g_out"] = nc.dram_tensor("g_out", [S, 1], F32, kind="ExternalOutput").ap()

    with tile.TileContext(nc) as tc:
        _build_body(nc, tc, io)
    nc.compile()
    _optimize_act_table_loads(nc)
    return nc


_PROGRAM = None


def get_program():
    global _PROGRAM
    if _PROGRAM is None:
        _PROGRAM = build_program()
    return _PROGRAM


def prep_inputs(inputs):
    """Build the 8 per-core input maps (token-rotated full inputs)."""
    f = lambda k: np.ascontiguousarray(np.asarray(inputs[k], dtype=np.float32))
    vis_T = f("vis").T.copy()          # [1024, 4096]
    pos_T = f("pos").T.copy()          # [3, 4096]
    grad = f("grad")                   # [4096, 1]
    shared = {}
    for k in ("B_fourier", "W_img", "b_img", "W_pos", "b_pos", "Wq", "bq", "Wk",
              "bk", "Wv", "bv", "Wo", "bo", "ln1_g", "ln1_b", "W_router",
              "b_router", "W1e", "b1e", "W2e", "b2e", "Wg1", "bg1", "lng_g",
              "lng_b", "Wg2", "bg2", "Wf1", "bf1", "Wf2", "bf2"):
        shared[k] = f(k)
    in_maps = []
    for c in range(NCORES):
        o = c * S
        m = dict(shared)
        m["visT"] = np.ascontiguousarray(np.roll(vis_T, -o, axis=1))
        m["posT"] = np.ascontiguousarray(np.roll(pos_T, -o, axis=1))
        m["gradT"] = np.ascontiguousarray(grad[o:o + S, 0][None, :])
        in_maps.append(m)
    return in_maps


def kernel(**inputs):
    nc = get_program()
    in_maps = prep_inputs(inputs)
    res = run_bass_kernel_spmd(nc, in_maps, core_ids=list(range(NCORES)))
    preds = np.concatenate([res.results[c]["preds_out"] for c in range(NCORES)], 0)
    g = np.concatenate([res.results[c]["g_out"] for c in range(NCORES)], 0)
    preds = preds.reshape(N, G, 2)
    mu = np.ascontiguousarray(preds[:, :, 0])
    theta = np.ascontiguousarray(preds[:, :, 1])
    return mu, theta, g
